# revision 3
# baseline (speedup 1.0000x reference)
"""Cluster-based contrastive loss on 8 Trainium2 NeuronCores.

Strategy: shard the C=50 cluster axis across 8 cores (7 slots/core, padded
to 56 global slots; 6 dummy slots carry weight 0).  Each core:
  - computes exact top-128 thresholds for its clusters (gpsimd kth_largest
    returns the 129th-largest value exactly),
  - finds per-partition top-8 values+indices (DVE max/max_index), ranks the
    survivors with a prefix-sum matmul and compacts them with a gpsimd
    local_scatter (one-hot matmul eliminated),
  - gathers the selected z_i / z_j rows from HBM (dma_gather),
  - normalizes (exp(-0.5*ln(|z|^2)) so the whole kernel uses one ACT
    function table) + transposes into a bf16 [128, D->partition] column
    table via a PE matmul against diag(w / ||z||),
  - AllGathers the table in 4 column-chunks (fired as soon as each chunk
    is built) so the collective overlaps the sim sweep,
  - sweeps flatT.T @ flatT supertile-major (1792 cols per supertile) with
    ACT exp (accum_out gives row sums free), subtracts the own-cluster
    block, and reduces log(neg) - log(pos) to one scalar.
The host sums the 8 per-core partial scalars.
"""

import sys

sys.path.insert(0, "/opt/trn_rl_repo")

import numpy as np

import concourse.bacc as bacc
import concourse.bass as bass
import concourse.mybir as mybir
from concourse import tile
from concourse.bass_utils import run_bass_kernel_spmd

F32 = mybir.dt.float32
BF16 = mybir.dt.bfloat16
I16 = mybir.dt.int16
U16 = mybir.dt.uint16
AF = mybir.ActivationFunctionType
ALU = mybir.AluOpType

B = 16384
D = 128
C = 50
K = 128
TEMP = 0.5
N_CORES = 8
SLOTS = 7                      # cluster slots per core
GSLOTS = N_CORES * SLOTS       # 56 global slots
TBL = GSLOTS * 2 * K           # 14336 columns in the padded table
LOCAL = SLOTS * 2 * K          # 1792 columns contributed per core
NBLK = SLOTS * 2               # 14 row blocks per core
N_DUMMY_COLS = (GSLOTS - C) * 2 * K   # 1536 zero columns in the table
# core k owns clusters [CBASE[k], CBASE[k] + CCNT[k])
CCNT = [7, 7, 6, 6, 6, 6, 6, 6]
CBASE = [0, 7, 14, 20, 26, 32, 38, 44]
QUANTILE = 1.0 - 127.5 / (B - 1)
# AllGather column chunks of the local table (fired as stage E completes
# each range); group k of the sweep table holds all 7 peers' chunk k.
CH_OFF = [0, 512, 1024, 1536]
CH_SZ = [512, 512, 512, 256]
CH_DONE_B = [3, 7, 11, 13]     # last stage-E block (2s+h) filling each chunk
GOFF = [1792, 5376, 8960, 12544]
ST = 1792                      # sweep supertile width; TBL = 8 * ST
NST = TBL // ST

_CACHE = {}


def _host_constants():
    lexcl = (np.arange(128)[:, None] < np.arange(128)[None, :]).astype(np.float32)
    ident = np.eye(128, dtype=np.float32)
    # rep16flat[0, 128*m + j] = (j % 16 == m): outer-product rows used to
    # redistribute the compacted [1, 896] index row into 16-partition wrap.
    rep16flat = np.zeros((1, 16 * 128), dtype=np.float32)
    for m in range(16):
        rep16flat[0, 128 * m + np.arange(m, 128, 16)] = 1.0
    # rankbase[p, 8c+t] = 128*c + t + 1  (cluster-global scatter slot, +1 so
    # invalid lanes become -1 after (x * valid) - 1)
    rb = (128 * np.arange(SLOTS)[:, None] + np.arange(8)[None, :] + 1).astype(
        np.float32
    )
    rankbase = np.broadcast_to(rb.reshape(1, SLOTS * 8), (128, SLOTS * 8)).copy()
    poff = (128.0 * np.arange(128, dtype=np.float32))[:, None].copy()
    return {
        "lexcl": lexcl,
        "ident": ident,
        "rep16flat": rep16flat,
        "rankbase": rankbase,
        "poff": poff,
    }


def _build_program(repeats=1):
    nc = bacc.Bacc(
        "TRN2", target_bir_lowering=False, debug=False, num_devices=N_CORES
    )

    probT = nc.dram_tensor("probT", [SLOTS, B], F32, kind="ExternalInput")
    z_i = nc.dram_tensor("z_i", [B, D], F32, kind="ExternalInput")
    z_j = nc.dram_tensor("z_j", [B, D], F32, kind="ExternalInput")
    wrow = nc.dram_tensor("wrow", [128, NBLK], F32, kind="ExternalInput")
    wfin = nc.dram_tensor("wfin", [1, NBLK], F32, kind="ExternalInput")
    lexcl = nc.dram_tensor("lexcl", [128, 128], F32, kind="ExternalInput")
    ident = nc.dram_tensor("ident", [128, 128], F32, kind="ExternalInput")
    rep16flat = nc.dram_tensor("rep16flat", [1, 2048], F32, kind="ExternalInput")
    rankbase = nc.dram_tensor("rankbase", [128, SLOTS * 8], F32, kind="ExternalInput")
    poff = nc.dram_tensor("poff", [128, 1], F32, kind="ExternalInput")
    outs = [
        nc.dram_tensor(f"partial{r}", [1, 1], F32, kind="ExternalOutput")
        for r in range(repeats)
    ]

    with tile.TileContext(nc) as tc:
        for r in range(repeats):
            _emit(
                nc, tc, probT, z_i, z_j, wrow, wfin, lexcl, ident, rep16flat,
                rankbase, poff, outs[r], rep=r,
            )

    nc.compile()
    return nc


def _emit(nc, tc, probT, z_i, z_j, wrow, wfin, lexcl, ident, rep16flat,
          rankbase, poff, out, rep=0):
    from contextlib import ExitStack

    R = f"r{rep}_"
    ctx = ExitStack()
    with ctx:
        const = ctx.enter_context(tc.tile_pool(name=R + "const", bufs=1))
        main = ctx.enter_context(tc.tile_pool(name=R + "main", bufs=1))
        scr = ctx.enter_context(tc.tile_pool(name=R + "scr", bufs=2))
        escr = ctx.enter_context(tc.tile_pool(name=R + "escr", bufs=4))
        setup_ctx = ExitStack()
        psum_tp = setup_ctx.enter_context(
            tc.tile_pool(name=R + "psum_tp", bufs=2, space="PSUM")
        )
        psum_sm = setup_ctx.enter_context(
            tc.tile_pool(name=R + "psum_sm", bufs=2, space="PSUM")
        )
        dram = ctx.enter_context(tc.tile_pool(name=R + "dram", bufs=1, space="DRAM"))

        # ---- constants -------------------------------------------------
        lexcl_sb = const.tile([128, 128], F32, tag="lexcl")
        ident_sb = const.tile([128, 128], F32, tag="ident")
        rep16f_sb = const.tile([1, 2048], F32, tag="rep16f")
        rankbase_sb = const.tile([128, SLOTS * 8], F32, tag="rankbase")
        poff_sb = const.tile([128, 1], F32, tag="poff")
        wrow_sb = const.tile([128, NBLK], F32, tag="wrow")
        wfin_sb = const.tile([1, NBLK], F32, tag="wfin")
        ones_p = const.tile([128, 1], F32, tag="ones_p")    # column of ones
        ones_r = const.tile([1, 128], F32, tag="ones_r")    # row of ones
        for dst, src in [
            (lexcl_sb, lexcl),
            (ident_sb, ident),
            (rep16f_sb, rep16flat),
            (rankbase_sb, rankbase),
            (poff_sb, poff),
            (wrow_sb, wrow),
            (wfin_sb, wfin),
        ]:
            nc.sync.dma_start(dst[:], src[:])
        nc.vector.memset(ones_p[:], 1.0)
        nc.vector.memset(ones_r[:], 1.0)

        # ---- stage A: prob + thresholds --------------------------------
        prob_sb = main.tile([128, SLOTS, 128], F32, tag="prob")
        # prob_sb[p, c, f] = probT[c, p*128 + f]
        nc.sync.dma_start(
            prob_sb[:], probT.ap().rearrange("c (p f) -> p c f", p=128)
        )
        taus = main.tile([1, 2 * SLOTS], F32, tag="taus")
        for c in range(SLOTS):
            nc.gpsimd.kth_largest(
                taus[0:1, 2 * c : 2 * c + 2],
                prob_sb[:, c, :],
                n_per_lane=128,
                k=K + 2,
                quantile=QUANTILE,
            )
        # broadcast tau (second output = 129th largest) to 128 partitions
        taub_ps = psum_sm.tile([128, SLOTS], F32, tag="sm")
        nc.tensor.matmul(taub_ps[:], ones_r[:], taus[0:1, 1 : 2 * SLOTS : 2])
        taub = main.tile([128, SLOTS], F32, tag="taub_sb")
        nc.vector.tensor_copy(taub[:], taub_ps[:])

        # ---- stage B: index extraction via max8 + local_scatter --------
        vals3 = main.tile([128, SLOTS, 8], F32, tag="vals3")
        idxs3 = main.tile([128, SLOTS, 8], U16, tag="idxs3")
        for c in range(SLOTS):
            nc.vector.max(vals3[:, c, :], prob_sb[:, c, :])
            nc.vector.max_index(idxs3[:, c, :], vals3[:, c, :], prob_sb[:, c, :])
        valid3 = scr.tile([128, SLOTS, 8], F32, tag="valid3")
        nc.vector.tensor_tensor(
            valid3[:],
            vals3[:],
            taub[:].rearrange("p (c o) -> p c o", o=1).to_broadcast([128, SLOTS, 8]),
            op=ALU.is_gt,
        )
        rowcnt = scr.tile([128, SLOTS], F32, tag="rowcnt")
        nc.vector.tensor_reduce(
            rowcnt[:], valid3[:], axis=mybir.AxisListType.X, op=ALU.add
        )
        rowoff_ps = psum_sm.tile([128, SLOTS], F32, tag="sm")
        nc.tensor.matmul(rowoff_ps[:], lexcl_sb[:], rowcnt[:])
        ranks = scr.tile([128, SLOTS, 8], F32, tag="ranks")
        nc.vector.tensor_tensor(
            ranks[:],
            rowoff_ps[:]
            .rearrange("p (c o) -> p c o", o=1)
            .to_broadcast([128, SLOTS, 8]),
            rankbase_sb[:].rearrange("p (c t) -> p c t", t=8),
            op=ALU.add,
        )
        nc.vector.tensor_tensor(ranks[:], ranks[:], valid3[:], op=ALU.mult)
        nc.vector.tensor_scalar_add(ranks[:], ranks[:], -1.0)
        rank16 = main.tile([128, SLOTS * 8], I16, tag="rank16")
        nc.vector.tensor_copy(rank16[:], ranks[:].rearrange("p c t -> p (c t)"))
        # global index = 128 * partition + local index
        gidxf = scr.tile([128, SLOTS * 8], F32, tag="gidxf")
        nc.vector.tensor_copy(gidxf[:], idxs3[:].rearrange("p c t -> p (c t)"))
        nc.vector.tensor_tensor(
            gidxf[:], gidxf[:], poff_sb[:].to_broadcast([128, SLOTS * 8]),
            op=ALU.add,
        )
        cand16 = main.tile([128, SLOTS * 8], I16, tag="cand16")
        nc.vector.tensor_copy(cand16[:], gidxf[:])
        evr16 = main.tile([128, SLOTS * 128], I16, tag="evr16")
        nc.gpsimd.local_scatter(
            evr16[:], cand16[:], rank16[:],
            channels=128, num_elems=SLOTS * 128, num_idxs=SLOTS * 8,
        )
        evrf = scr.tile([128, SLOTS * 128], F32, tag="evrf")
        nc.vector.tensor_copy(evrf[:], evr16[:])
        # compact to a single [1, 896] row (each column has one nonzero)
        allidx_ps = psum_sm.tile([1, SLOTS * 128], F32, tag="smwide")
        nc.tensor.matmul(allidx_ps[:, 0:512], ones_p[:], evrf[:, 0:512])
        nc.tensor.matmul(
            allidx_ps[:, 512 : SLOTS * 128], ones_p[:], evrf[:, 512 : SLOTS * 128]
        )
        allidx = main.tile([1, SLOTS * 128], F32, tag="allidx")
        nc.scalar.copy(allidx[:], allidx_ps[:])
        # redistribute into the 16-partition wrap dma_gather expects
        widx_ps = psum_sm.tile([128, 56], F32, tag="sm")
        av = allidx[:].rearrange("p (s m) -> p m s", m=16)
        for m in range(16):
            nc.tensor.matmul(
                widx_ps[:],
                rep16f_sb[0:1, 128 * m : 128 * (m + 1)],
                av[0:1, m, :],
                start=(m == 0),
                stop=(m == 15),
            )
        idxs_i16 = main.tile([128, 56], I16, tag="idxs")
        nc.vector.tensor_copy(idxs_i16[:], widx_ps[:])

        # ---- stage D: gather selected rows -----------------------------
        gi = main.tile([128, SLOTS, 128], F32, tag="gi")
        gj = main.tile([128, SLOTS, 128], F32, tag="gj")
        for g_sb, z in ((gi, z_i), (gj, z_j)):
            nc.gpsimd.dma_gather(
                g_sb[:],
                z.ap(),
                idxs_i16[:],
                num_idxs=SLOTS * 128,
                num_idxs_reg=SLOTS * 128,
                elem_size=D,
            )

        # ---- stage E: normalize + transpose into bf16 table ------------
        # 1/|z| = exp(-0.5 * ln(|z|^2)) keeps every activation in the
        # natural_log_exp_and_others table (no Sqrt -> no table reloads).
        sqs = main.tile([128, NBLK], F32, tag="sqs")
        for b in range(NBLK):
            s, h = b // 2, b % 2
            src = (gi if h == 0 else gj)[:, s, :]
            trash = scr.tile([128, 128], F32, tag="trash")
            nc.scalar.activation(
                trash[:], src, AF.Square, accum_out=sqs[:, b : b + 1]
            )
        lnv = scr.tile([128, NBLK], F32, tag="lnv")
        nc.scalar.activation(lnv[:], sqs[:], AF.Ln)
        rnw = main.tile([128, NBLK], F32, tag="rnw")
        nc.scalar.activation(rnw[:], lnv[:], AF.Exp, scale=-0.5)
        nc.vector.tensor_tensor(rnw[:], rnw[:], wrow_sb[:], op=ALU.mult)

        agin = [
            dram.tile([128, CH_SZ[k]], BF16, name=f"agin{k}") for k in range(4)
        ]
        agout = [
            dram.tile(
                [N_CORES * 128, CH_SZ[k]], BF16, addr_space="Shared",
                name=f"agout{k}",
            )
            for k in range(4)
        ]
        flatT = main.tile([128, TBL], BF16, tag="flatT")
        ag_fired = 0
        for b in range(NBLK):
            s, h = b // 2, b % 2
            src = (gi if h == 0 else gj)[:, s, :]
            diag = scr.tile([128, 128], F32, tag="diag")
            nc.vector.tensor_tensor(
                diag[:], ident_sb[:], rnw[:, b : b + 1].to_broadcast([128, 128]),
                op=ALU.mult,
            )
            tp_ps = psum_tp.tile([128, 128], F32, tag="tp")
            nc.tensor.matmul(tp_ps[:], src, diag[:])
            nc.scalar.copy(
                flatT[:, 256 * s + 128 * h : 256 * s + 128 * h + 128], tp_ps[:]
            )
            # fire AllGather chunks as soon as their columns are built
            while ag_fired < 4 and b == CH_DONE_B[ag_fired]:
                k = ag_fired
                nc.sync.dma_start(
                    agin[k][:], flatT[:, CH_OFF[k] : CH_OFF[k] + CH_SZ[k]]
                )
                nc.gpsimd.collective_compute(
                    "AllGather",
                    ALU.bypass,
                    replica_groups=[list(range(N_CORES))],
                    ins=[agin[k].opt()],
                    outs=[agout[k].opt()],
                )
                ag_fired += 1

        # ---- stage F: rotated reload of peer chunks --------------------
        pid = nc.partition_id()
        for k in range(4):
            for j in range(1, N_CORES):
                rj = (pid + j) & 7
                dst = GOFF[k] + (j - 1) * CH_SZ[k]
                nc.sync.dma_start(
                    flatT[:, dst : dst + CH_SZ[k]],
                    agout[k][bass.ds(rj * 128, 128), :],
                )

        # ---- stage G: sim sweep (supertile-major) ----------------------
        setup_ctx.close()
        psum_sim = ctx.enter_context(
            tc.tile_pool(name=R + "psum_sim", bufs=2, space="PSUM")
        )
        partials = main.tile([128, NBLK, NST], F32, tag="partials")
        own_t = main.tile([128, NBLK], F32, tag="own_t")
        pos_t = main.tile([128, NBLK], F32, tag="pos_t")
        for st in range(NST):
            for b in range(NBLK):
                s, h = b // 2, b % 2
                lhsT = flatT[:, 256 * s + 128 * h : 256 * s + 128 * h + 128]
                sim_ps = psum_sim.tile([128, 2048], F32, tag="sim")
                for q0, qs in ((0, 512), (512, 512), (1024, 512), (1536, 256)):
                    nc.tensor.matmul(
                        sim_ps[:, q0 : q0 + qs],
                        lhsT,
                        flatT[:, ST * st + q0 : ST * st + q0 + qs],
                    )
                e_sb = escr.tile([128, ST], BF16, tag="e")
                nc.scalar.activation(
                    e_sb[:],
                    sim_ps[:, 0:ST],
                    AF.Exp,
                    scale=1.0 / TEMP,
                    accum_out=partials[:, b, st : st + 1],
                )
                if st == 0:
                    off = 256 * s
                    nc.vector.tensor_reduce(
                        pos_t[:, b : b + 1],
                        e_sb[:, off : off + 128],
                        axis=mybir.AxisListType.X,
                        op=ALU.add,
                    )
                    nc.vector.tensor_reduce(
                        own_t[:, b : b + 1],
                        e_sb[:, off : off + 256],
                        axis=mybir.AxisListType.X,
                        op=ALU.add,
                    )

        # ---- stage H: reduce to one scalar -----------------------------
        totals = main.tile([128, NBLK], F32, tag="totals")
        nc.vector.tensor_reduce(
            totals[:], partials[:], axis=mybir.AxisListType.X, op=ALU.add
        )
        neg = scr.tile([128, NBLK], F32, tag="neg")
        nc.vector.scalar_tensor_tensor(
            neg[:], totals[:], float(-N_DUMMY_COLS), own_t[:],
            op0=ALU.add, op1=ALU.subtract,
        )
        lnn = scr.tile([128, NBLK], F32, tag="lnn")
        lnp = scr.tile([128, NBLK], F32, tag="lnp")
        nc.scalar.activation(lnn[:], neg[:], AF.Ln)
        nc.scalar.activation(lnp[:], pos_t[:], AF.Ln)
        lrows = main.tile([128, NBLK], F32, tag="lrows")
        nc.vector.tensor_sub(lrows[:], lnn[:], lnp[:])
        fin_ps = psum_sim.tile([1, NBLK], F32, tag="sim")
        nc.tensor.matmul(fin_ps[:], ones_p[:], lrows[:])
        fin_sb = main.tile([1, NBLK], F32, tag="fin_sb")
        nc.vector.tensor_tensor(fin_sb[:], fin_ps[:], wfin_sb[:], op=ALU.mult)
        out_sb = main.tile([1, 1], F32, tag="out_sb")
        nc.vector.tensor_reduce(
            out_sb[:], fin_sb[:], axis=mybir.AxisListType.X, op=ALU.add
        )
        nc.vector.tensor_scalar_mul(out_sb[:], out_sb[:], 1.0 / (2 * K * C))
        nc.sync.dma_start(out[:], out_sb[:])


def _per_core_inputs(prob, z_i, z_j):
    consts = _host_constants()
    maps = []
    for k in range(N_CORES):
        ncl = CCNT[k]
        cols = list(range(CBASE[k], CBASE[k] + ncl))
        cols = cols + [CBASE[k]] * (SLOTS - ncl)  # dummy slots reuse first col
        pT = np.ascontiguousarray(prob[:, cols].T)  # [SLOTS, B]
        w = np.array([1.0] * ncl + [0.0] * (SLOTS - ncl), dtype=np.float32)
        wrow = np.broadcast_to(
            np.repeat(w, 2)[None, :], (128, NBLK)
        ).copy()  # [128, 14]
        wfin = np.repeat(w, 2)[None, :].astype(np.float32).copy()  # [1, 14]
        m = {
            "probT": pT,
            "z_i": z_i,
            "z_j": z_j,
            "wrow": wrow,
            "wfin": wfin,
        }
        m.update(consts)
        maps.append(m)
    return maps


def kernel(prob, z_i, z_j):
    if "nc" not in _CACHE:
        _CACHE["nc"] = _build_program()
    nc = _CACHE["nc"]
    in_maps = _per_core_inputs(
        np.asarray(prob, dtype=np.float32),
        np.ascontiguousarray(z_i, dtype=np.float32),
        np.ascontiguousarray(z_j, dtype=np.float32),
    )
    res = run_bass_kernel_spmd(nc, in_maps, list(range(N_CORES)))
    total = np.float32(0.0)
    for r in res.results:
        total += r["partial0"][0, 0]
    return np.asarray(total, dtype=np.float32)


# revision 8
# speedup vs baseline: 1.0161x; 1.0161x over previous
"""Cluster-based contrastive loss on 8 Trainium2 NeuronCores.

Strategy: shard the C=50 cluster axis across 8 cores (7 slots/core, padded
to 56 global slots; 6 dummy slots carry weight 0).  Each core:
  - computes exact top-128 thresholds for its clusters (gpsimd kth_largest
    returns the 129th-largest value exactly),
  - finds per-partition top-8 values+indices (DVE max/max_index), ranks the
    survivors with a prefix-sum matmul and compacts them with a gpsimd
    local_scatter (one-hot matmul eliminated),
  - gathers the selected z_i / z_j rows from HBM (dma_gather),
  - normalizes (exp(-0.5*ln(|z|^2)) so the whole kernel uses one ACT
    function table) + transposes into a bf16 [128, D->partition] column
    table via a PE matmul against diag(w / ||z||),
  - AllGathers the table in 4 column-chunks (fired as soon as each chunk
    is built) so the collective overlaps the sim sweep,
  - sweeps flatT.T @ flatT supertile-major (1792 cols per supertile) with
    ACT exp (accum_out gives row sums free), subtracts the own-cluster
    block, and reduces log(neg) - log(pos) to one scalar.
The host sums the 8 per-core partial scalars.
"""

import sys

sys.path.insert(0, "/opt/trn_rl_repo")

import numpy as np

import concourse.bacc as bacc
import concourse.bass as bass
import concourse.mybir as mybir
from concourse import tile
from concourse.bass_utils import run_bass_kernel_spmd

F32 = mybir.dt.float32
BF16 = mybir.dt.bfloat16
I16 = mybir.dt.int16
U16 = mybir.dt.uint16
AF = mybir.ActivationFunctionType
ALU = mybir.AluOpType

B = 16384
D = 128
C = 50
K = 128
TEMP = 0.5
N_CORES = 8
SLOTS = 7                      # cluster slots per core
GSLOTS = N_CORES * SLOTS       # 56 global slots
TBL = GSLOTS * 2 * K           # 14336 columns in the padded table
LOCAL = SLOTS * 2 * K          # 1792 columns contributed per core
NBLK = SLOTS * 2               # 14 row blocks per core
N_DUMMY_COLS = (GSLOTS - C) * 2 * K   # 1536 zero columns in the table
# core k owns clusters [CBASE[k], CBASE[k] + CCNT[k])
CCNT = [7, 7, 6, 6, 6, 6, 6, 6]
CBASE = [0, 7, 14, 20, 26, 32, 38, 44]
QUANTILE = 1.0 - 127.5 / (B - 1)
# AllGather column chunks of the local table (fired as stage E completes
# each range); group k of the sweep table holds all 7 peers' chunk k.
CH_OFF = [0, 512, 1024, 1536]
CH_SZ = [512, 512, 512, 256]
CH_DONE_B = [3, 7, 11, 13]     # last stage-E block (2s+h) filling each chunk
GOFF = [1792, 5376, 8960, 12544]
ST = 1792                      # sweep supertile width; TBL = 8 * ST
NST = TBL // ST

_CACHE = {}


def _host_constants():
    lexcl = (np.arange(128)[:, None] < np.arange(128)[None, :]).astype(np.float32)
    ident = np.eye(128, dtype=np.float32)
    # rep16flat[0, 128*m + j] = (j % 16 == m): outer-product rows used to
    # redistribute the compacted [1, 896] index row into 16-partition wrap.
    rep16flat = np.zeros((1, 16 * 128), dtype=np.float32)
    for m in range(16):
        rep16flat[0, 128 * m + np.arange(m, 128, 16)] = 1.0
    # rankbase[p, 8c+t] = 128*c + t + 1  (cluster-global scatter slot, +1 so
    # invalid lanes become -1 after (x * valid) - 1)
    rb = (128 * np.arange(SLOTS)[:, None] + np.arange(8)[None, :] + 1).astype(
        np.float32
    )
    rankbase = np.broadcast_to(rb.reshape(1, SLOTS * 8), (128, SLOTS * 8)).copy()
    poff = (128.0 * np.arange(128, dtype=np.float32))[:, None].copy()
    return {
        "lexcl": lexcl,
        "ident": ident,
        "rep16flat": rep16flat,
        "rankbase": rankbase,
        "poff": poff,
    }


def _build_program(repeats=1, variant="full"):
    nc = bacc.Bacc(
        "TRN2", target_bir_lowering=False, debug=False, num_devices=N_CORES
    )

    probT = nc.dram_tensor("probT", [SLOTS, B], F32, kind="ExternalInput")
    z_i = nc.dram_tensor("z_i", [B, D], F32, kind="ExternalInput")
    z_j = nc.dram_tensor("z_j", [B, D], F32, kind="ExternalInput")
    wrow = nc.dram_tensor("wrow", [128, NBLK], F32, kind="ExternalInput")
    wfin = nc.dram_tensor("wfin", [1, NBLK], F32, kind="ExternalInput")
    lexcl = nc.dram_tensor("lexcl", [128, 128], F32, kind="ExternalInput")
    ident = nc.dram_tensor("ident", [128, 128], F32, kind="ExternalInput")
    rep16flat = nc.dram_tensor("rep16flat", [1, 2048], F32, kind="ExternalInput")
    rankbase = nc.dram_tensor("rankbase", [128, SLOTS * 8], F32, kind="ExternalInput")
    poff = nc.dram_tensor("poff", [128, 1], F32, kind="ExternalInput")
    outs = [
        nc.dram_tensor(f"partial{r}", [1, 1], F32, kind="ExternalOutput")
        for r in range(repeats)
    ]

    with tile.TileContext(nc) as tc:
        for r in range(repeats):
            _emit(
                nc, tc, probT, z_i, z_j, wrow, wfin, lexcl, ident, rep16flat,
                rankbase, poff, outs[r], rep=r, variant=variant,
            )

    nc.compile()
    return nc


def _emit(nc, tc, probT, z_i, z_j, wrow, wfin, lexcl, ident, rep16flat,
          rankbase, poff, out, rep=0, variant="full"):
    from contextlib import ExitStack

    R = f"r{rep}_"
    ctx = ExitStack()
    with ctx:
        const = ctx.enter_context(tc.tile_pool(name=R + "const", bufs=1))
        main = ctx.enter_context(tc.tile_pool(name=R + "main", bufs=1))
        scr = ctx.enter_context(tc.tile_pool(name=R + "scr", bufs=2))
        escr = ctx.enter_context(tc.tile_pool(name=R + "escr", bufs=4))
        setup_ctx = ExitStack()
        psum_tp = setup_ctx.enter_context(
            tc.tile_pool(name=R + "psum_tp", bufs=2, space="PSUM")
        )
        psum_sm = setup_ctx.enter_context(
            tc.tile_pool(name=R + "psum_sm", bufs=2, space="PSUM")
        )
        dram = ctx.enter_context(tc.tile_pool(name=R + "dram", bufs=1, space="DRAM"))

        # ---- constants -------------------------------------------------
        lexcl_sb = const.tile([128, 128], F32, tag="lexcl")
        ident_sb = const.tile([128, 128], F32, tag="ident")
        rep16f_sb = const.tile([1, 2048], F32, tag="rep16f")
        rankbase_sb = const.tile([128, SLOTS * 8], F32, tag="rankbase")
        poff_sb = const.tile([128, 1], F32, tag="poff")
        wrow_sb = const.tile([128, NBLK], F32, tag="wrow")
        wfin_sb = const.tile([1, NBLK], F32, tag="wfin")
        ones_p = const.tile([128, 1], F32, tag="ones_p")    # column of ones
        ones_r = const.tile([1, 128], F32, tag="ones_r")    # row of ones
        for dst, src in [
            (lexcl_sb, lexcl),
            (ident_sb, ident),
            (rep16f_sb, rep16flat),
            (rankbase_sb, rankbase),
            (poff_sb, poff),
            (wrow_sb, wrow),
            (wfin_sb, wfin),
        ]:
            nc.sync.dma_start(dst[:], src[:])
        nc.vector.memset(ones_p[:], 1.0)
        nc.vector.memset(ones_r[:], 1.0)

        # ---- stage A: prob + thresholds --------------------------------
        prob_sb = main.tile([128, SLOTS, 128], F32, tag="prob")
        # prob_sb[p, c, f] = probT[c, p*128 + f]
        nc.sync.dma_start(
            prob_sb[:], probT.ap().rearrange("c (p f) -> p c f", p=128)
        )
        taus = main.tile([1, 2 * SLOTS], F32, tag="taus")
        for c in range(SLOTS):
            nc.gpsimd.kth_largest(
                taus[0:1, 2 * c : 2 * c + 2],
                prob_sb[:, c, :],
                n_per_lane=128,
                k=K + 2,
                quantile=QUANTILE,
            )
        # broadcast tau (second output = 129th largest) to 128 partitions
        taub_ps = psum_sm.tile([128, SLOTS], F32, tag="sm")
        nc.tensor.matmul(taub_ps[:], ones_r[:], taus[0:1, 1 : 2 * SLOTS : 2])
        taub = main.tile([128, SLOTS], F32, tag="taub_sb")
        nc.vector.tensor_copy(taub[:], taub_ps[:])

        # ---- stage B: index extraction via max8 + local_scatter --------
        vals3 = main.tile([128, SLOTS, 8], F32, tag="vals3")
        idxs3 = main.tile([128, SLOTS, 8], U16, tag="idxs3")
        for c in range(SLOTS):
            nc.vector.max(vals3[:, c, :], prob_sb[:, c, :])
            nc.vector.max_index(idxs3[:, c, :], vals3[:, c, :], prob_sb[:, c, :])
        valid3 = scr.tile([128, SLOTS, 8], F32, tag="valid3")
        nc.vector.tensor_tensor(
            valid3[:],
            vals3[:],
            taub[:].rearrange("p (c o) -> p c o", o=1).to_broadcast([128, SLOTS, 8]),
            op=ALU.is_gt,
        )
        rowcnt = scr.tile([128, SLOTS], F32, tag="rowcnt")
        nc.vector.tensor_reduce(
            rowcnt[:], valid3[:], axis=mybir.AxisListType.X, op=ALU.add
        )
        rowoff_ps = psum_sm.tile([128, SLOTS], F32, tag="sm")
        nc.tensor.matmul(rowoff_ps[:], lexcl_sb[:], rowcnt[:])
        ranks = scr.tile([128, SLOTS, 8], F32, tag="ranks")
        nc.vector.tensor_tensor(
            ranks[:],
            rowoff_ps[:]
            .rearrange("p (c o) -> p c o", o=1)
            .to_broadcast([128, SLOTS, 8]),
            rankbase_sb[:].rearrange("p (c t) -> p c t", t=8),
            op=ALU.add,
        )
        nc.vector.tensor_tensor(ranks[:], ranks[:], valid3[:], op=ALU.mult)
        nc.vector.tensor_scalar_add(ranks[:], ranks[:], -1.0)
        rank16 = main.tile([128, SLOTS * 8], I16, tag="rank16")
        nc.vector.tensor_copy(rank16[:], ranks[:].rearrange("p c t -> p (c t)"))
        # global index = 128 * partition + local index
        gidxf = scr.tile([128, SLOTS * 8], F32, tag="gidxf")
        nc.vector.tensor_copy(gidxf[:], idxs3[:].rearrange("p c t -> p (c t)"))
        nc.vector.tensor_tensor(
            gidxf[:], gidxf[:], poff_sb[:].to_broadcast([128, SLOTS * 8]),
            op=ALU.add,
        )
        cand16 = main.tile([128, SLOTS * 8], I16, tag="cand16")
        nc.vector.tensor_copy(cand16[:], gidxf[:])
        evr16 = main.tile([128, SLOTS * 128], I16, tag="evr16")
        nc.gpsimd.local_scatter(
            evr16[:], cand16[:], rank16[:],
            channels=128, num_elems=SLOTS * 128, num_idxs=SLOTS * 8,
        )
        evrf = scr.tile([128, SLOTS * 128], F32, tag="evrf")
        nc.vector.tensor_copy(evrf[:], evr16[:])
        # compact to a single [1, 896] row (each column has one nonzero)
        allidx_ps = psum_sm.tile([1, SLOTS * 128], F32, tag="smwide")
        nc.tensor.matmul(allidx_ps[:, 0:512], ones_p[:], evrf[:, 0:512])
        nc.tensor.matmul(
            allidx_ps[:, 512 : SLOTS * 128], ones_p[:], evrf[:, 512 : SLOTS * 128]
        )
        allidx = main.tile([1, SLOTS * 128], F32, tag="allidx")
        nc.scalar.copy(allidx[:], allidx_ps[:])
        # redistribute into the 16-partition wrap dma_gather expects
        widx_ps = psum_sm.tile([128, 56], F32, tag="sm")
        av = allidx[:].rearrange("p (s m) -> p m s", m=16)
        for m in range(16):
            nc.tensor.matmul(
                widx_ps[:],
                rep16f_sb[0:1, 128 * m : 128 * (m + 1)],
                av[0:1, m, :],
                start=(m == 0),
                stop=(m == 15),
            )
        idxs_i16 = main.tile([128, 56], I16, tag="idxs")
        nc.vector.tensor_copy(idxs_i16[:], widx_ps[:])

        # ---- stage D: gather selected rows -----------------------------
        gi = main.tile([128, SLOTS, 128], F32, tag="gi")
        gj = main.tile([128, SLOTS, 128], F32, tag="gj")
        for g_sb, z in ((gi, z_i), (gj, z_j)):
            nc.gpsimd.dma_gather(
                g_sb[:],
                z.ap(),
                idxs_i16[:],
                num_idxs=SLOTS * 128,
                num_idxs_reg=SLOTS * 128,
                elem_size=D,
            )

        # ---- stage E: normalize + transpose into bf16 table ------------
        # 1/|z| = exp(-0.5 * ln(|z|^2)) keeps every activation in the
        # natural_log_exp_and_others table (no Sqrt -> no table reloads).
        sqs = main.tile([128, NBLK], F32, tag="sqs")
        for b in range(NBLK):
            s, h = b // 2, b % 2
            src = (gi if h == 0 else gj)[:, s, :]
            trash = scr.tile([128, 128], F32, tag="trash")
            nc.scalar.activation(
                trash[:], src, AF.Square, accum_out=sqs[:, b : b + 1]
            )
        lnv = scr.tile([128, NBLK], F32, tag="lnv")
        nc.scalar.activation(lnv[:], sqs[:], AF.Ln)
        rnw = main.tile([128, NBLK], F32, tag="rnw")
        nc.scalar.activation(rnw[:], lnv[:], AF.Exp, scale=-0.5)
        nc.vector.tensor_tensor(rnw[:], rnw[:], wrow_sb[:], op=ALU.mult)

        agin = [
            dram.tile([128, CH_SZ[k]], BF16, name=f"agin{k}") for k in range(4)
        ]
        agout = [
            dram.tile(
                [N_CORES * 128, CH_SZ[k]], BF16, addr_space="Shared",
                name=f"agout{k}",
            )
            for k in range(4)
        ]
        flatT = main.tile([128, TBL], BF16, tag="flatT")
        ag_fired = 0
        for b in range(NBLK):
            s, h = b // 2, b % 2
            src = (gi if h == 0 else gj)[:, s, :]
            diag = scr.tile([128, 128], F32, tag="diag")
            nc.vector.tensor_tensor(
                diag[:], ident_sb[:], rnw[:, b : b + 1].to_broadcast([128, 128]),
                op=ALU.mult,
            )
            tp_ps = psum_tp.tile([128, 128], F32, tag="tp")
            nc.tensor.matmul(tp_ps[:], src, diag[:])
            nc.scalar.copy(
                flatT[:, 256 * s + 128 * h : 256 * s + 128 * h + 128], tp_ps[:]
            )
            # fire AllGather chunks as soon as their columns are built
            while ag_fired < 4 and b == CH_DONE_B[ag_fired]:
                k = ag_fired
                if variant != "noag":
                    nc.sync.dma_start(
                        agin[k][:], flatT[:, CH_OFF[k] : CH_OFF[k] + CH_SZ[k]]
                    )
                    nc.gpsimd.collective_compute(
                        "AllGather",
                        ALU.bypass,
                        replica_groups=[list(range(N_CORES))],
                        ins=[agin[k].opt()],
                        outs=[agout[k].opt()],
                    )
                ag_fired += 1

        # ---- stage F: rotated reload of peer chunks --------------------
        if variant != "noag":
            pid = nc.partition_id()
            for k in range(4):
                for j in range(1, N_CORES):
                    rj = (pid + j) & 7
                    dst = GOFF[k] + (j - 1) * CH_SZ[k]
                    nc.sync.dma_start(
                        flatT[:, dst : dst + CH_SZ[k]],
                        agout[k][bass.ds(rj * 128, 128), :],
                    )
        else:
            # fill peer columns locally (wrong numerics, same sweep timing)
            for k in range(4):
                for j in range(1, N_CORES):
                    dst = GOFF[k] + (j - 1) * CH_SZ[k]
                    nc.sync.dma_start(
                        flatT[:, dst : dst + CH_SZ[k]],
                        flatT[:, CH_OFF[k] : CH_OFF[k] + CH_SZ[k]],
                    )

        # ---- stage G: sim sweep (supertile-major) ----------------------
        setup_ctx.close()
        psum_sim = ctx.enter_context(
            tc.tile_pool(name=R + "psum_sim", bufs=2, space="PSUM")
        )
        if variant == "nosweep":
            nos = main.tile([1, 1], F32, tag="nos")
            nc.vector.tensor_reduce(
                nos[:], flatT[0:1, :], axis=mybir.AxisListType.X, op=ALU.add
            )
            nc.sync.dma_start(out[:], nos[:])
            return
        partials = main.tile([128, NBLK, NST], F32, tag="partials")
        own_t = main.tile([128, NBLK], F32, tag="own_t")
        pos_t = main.tile([128, NBLK], F32, tag="pos_t")
        for st in range(NST):
            for b in range(NBLK):
                s, h = b // 2, b % 2
                lhsT = flatT[:, 256 * s + 128 * h : 256 * s + 128 * h + 128]
                sim_ps = psum_sim.tile([128, 2048], F32, tag="sim")
                for q0, qs in ((0, 512), (512, 512), (1024, 512), (1536, 256)):
                    nc.tensor.matmul(
                        sim_ps[:, q0 : q0 + qs],
                        lhsT,
                        flatT[:, ST * st + q0 : ST * st + q0 + qs],
                    )
                e_sb = escr.tile([128, ST], BF16, tag="e")
                nc.scalar.activation(
                    e_sb[:],
                    sim_ps[:, 0:ST],
                    AF.Exp,
                    scale=1.0 / TEMP,
                    accum_out=partials[:, b, st : st + 1],
                )
                if st == 0:
                    off = 256 * s
                    nc.vector.tensor_reduce(
                        pos_t[:, b : b + 1],
                        e_sb[:, off : off + 128],
                        axis=mybir.AxisListType.X,
                        op=ALU.add,
                    )
                    nc.vector.tensor_reduce(
                        own_t[:, b : b + 1],
                        e_sb[:, off : off + 256],
                        axis=mybir.AxisListType.X,
                        op=ALU.add,
                    )

        # ---- stage H: reduce to one scalar -----------------------------
        totals = main.tile([128, NBLK], F32, tag="totals")
        nc.vector.tensor_reduce(
            totals[:], partials[:], axis=mybir.AxisListType.X, op=ALU.add
        )
        neg = scr.tile([128, NBLK], F32, tag="neg")
        nc.vector.scalar_tensor_tensor(
            neg[:], totals[:], float(-N_DUMMY_COLS), own_t[:],
            op0=ALU.add, op1=ALU.subtract,
        )
        lnn = scr.tile([128, NBLK], F32, tag="lnn")
        lnp = scr.tile([128, NBLK], F32, tag="lnp")
        nc.scalar.activation(lnn[:], neg[:], AF.Ln)
        nc.scalar.activation(lnp[:], pos_t[:], AF.Ln)
        lrows = main.tile([128, NBLK], F32, tag="lrows")
        nc.vector.tensor_sub(lrows[:], lnn[:], lnp[:])
        fin_ps = psum_sim.tile([1, NBLK], F32, tag="sim")
        nc.tensor.matmul(fin_ps[:], ones_p[:], lrows[:])
        fin_sb = main.tile([1, NBLK], F32, tag="fin_sb")
        nc.vector.tensor_tensor(fin_sb[:], fin_ps[:], wfin_sb[:], op=ALU.mult)
        out_sb = main.tile([1, 1], F32, tag="out_sb")
        nc.vector.tensor_reduce(
            out_sb[:], fin_sb[:], axis=mybir.AxisListType.X, op=ALU.add
        )
        nc.vector.tensor_scalar_mul(out_sb[:], out_sb[:], 1.0 / (2 * K * C))
        nc.sync.dma_start(out[:], out_sb[:])


def _per_core_inputs(prob, z_i, z_j):
    consts = _host_constants()
    maps = []
    for k in range(N_CORES):
        ncl = CCNT[k]
        cols = list(range(CBASE[k], CBASE[k] + ncl))
        cols = cols + [CBASE[k]] * (SLOTS - ncl)  # dummy slots reuse first col
        pT = np.ascontiguousarray(prob[:, cols].T)  # [SLOTS, B]
        w = np.array([1.0] * ncl + [0.0] * (SLOTS - ncl), dtype=np.float32)
        wrow = np.broadcast_to(
            np.repeat(w, 2)[None, :], (128, NBLK)
        ).copy()  # [128, 14]
        wfin = np.repeat(w, 2)[None, :].astype(np.float32).copy()  # [1, 14]
        m = {
            "probT": pT,
            "z_i": z_i,
            "z_j": z_j,
            "wrow": wrow,
            "wfin": wfin,
        }
        m.update(consts)
        maps.append(m)
    return maps


def kernel(prob, z_i, z_j):
    if "nc" not in _CACHE:
        _CACHE["nc"] = _build_program()
    nc = _CACHE["nc"]
    in_maps = _per_core_inputs(
        np.asarray(prob, dtype=np.float32),
        np.ascontiguousarray(z_i, dtype=np.float32),
        np.ascontiguousarray(z_j, dtype=np.float32),
    )
    res = run_bass_kernel_spmd(nc, in_maps, list(range(N_CORES)))
    total = np.float32(0.0)
    for r in res.results:
        total += r["partial0"][0, 0]
    return np.asarray(total, dtype=np.float32)


# revision 9
# speedup vs baseline: 1.0319x; 1.0155x over previous
"""Cluster-based contrastive loss on 8 Trainium2 NeuronCores.

Strategy: shard the C=50 cluster axis across 8 cores (7 slots/core, padded
to 56 global slots; 6 dummy slots carry weight 0).  Each core:
  - computes exact top-128 thresholds for its clusters (gpsimd kth_largest
    returns the 129th-largest value exactly),
  - finds per-partition top-8 values+indices (DVE max/max_index), ranks the
    survivors with a prefix-sum matmul and compacts them with a gpsimd
    local_scatter (one-hot matmul eliminated),
  - gathers the selected z_i / z_j rows from HBM (dma_gather),
  - normalizes (exp(-0.5*ln(|z|^2)) so the whole kernel uses one ACT
    function table) + transposes into a bf16 [128, D->partition] column
    table via a PE matmul against diag(w / ||z||),
  - AllGathers the table in 4 column-chunks (fired as soon as each chunk
    is built) so the collective overlaps the sim sweep,
  - sweeps flatT.T @ flatT supertile-major (1792 cols per supertile) with
    ACT exp (accum_out gives row sums free), subtracts the own-cluster
    block, and reduces log(neg) - log(pos) to one scalar.
The host sums the 8 per-core partial scalars.
"""

import sys

sys.path.insert(0, "/opt/trn_rl_repo")

import numpy as np

import concourse.bacc as bacc
import concourse.bass as bass
import concourse.mybir as mybir
from concourse import tile
from concourse.bass_utils import run_bass_kernel_spmd

F32 = mybir.dt.float32
BF16 = mybir.dt.bfloat16
I16 = mybir.dt.int16
U16 = mybir.dt.uint16
AF = mybir.ActivationFunctionType
ALU = mybir.AluOpType

B = 16384
D = 128
C = 50
K = 128
TEMP = 0.5
N_CORES = 8
SLOTS = 7                      # cluster slots per core
GSLOTS = N_CORES * SLOTS       # 56 global slots
TBL = GSLOTS * 2 * K           # 14336 columns in the padded table
LOCAL = SLOTS * 2 * K          # 1792 columns contributed per core
NBLK = SLOTS * 2               # 14 row blocks per core
N_DUMMY_COLS = (GSLOTS - C) * 2 * K   # 1536 zero columns in the table
# core k owns clusters [CBASE[k], CBASE[k] + CCNT[k])
CCNT = [7, 7, 6, 6, 6, 6, 6, 6]
CBASE = [0, 7, 14, 20, 26, 32, 38, 44]
QUANTILE = 1.0 - 127.5 / (B - 1)
# AllGather column chunks of the local table (fired as stage E completes
# each range); group k of the sweep table holds all 7 peers' chunk k.
CH_OFF = [0, 512, 1024, 1536]
CH_SZ = [512, 512, 512, 256]
CH_DONE_B = [3, 7, 11, 13]     # last stage-E block (2s+h) filling each chunk
GOFF = [1792, 5376, 8960, 12544]
ST = 1792                      # sweep supertile width; TBL = 8 * ST
NST = TBL // ST

_CACHE = {}


def _host_constants():
    lexcl = (np.arange(128)[:, None] < np.arange(128)[None, :]).astype(np.float32)
    ident = np.eye(128, dtype=np.float32)
    # rep16flat[0, 128*m + j] = (j % 16 == m): outer-product rows used to
    # redistribute the compacted [1, 896] index row into 16-partition wrap.
    rep16flat = np.zeros((1, 16 * 128), dtype=np.float32)
    for m in range(16):
        rep16flat[0, 128 * m + np.arange(m, 128, 16)] = 1.0
    # rankbase[p, 8c+t] = 128*c + t + 1  (cluster-global scatter slot, +1 so
    # invalid lanes become -1 after (x * valid) - 1)
    rb = (128 * np.arange(SLOTS)[:, None] + np.arange(8)[None, :] + 1).astype(
        np.float32
    )
    rankbase = np.broadcast_to(rb.reshape(1, SLOTS * 8), (128, SLOTS * 8)).copy()
    poff = (128.0 * np.arange(128, dtype=np.float32))[:, None].copy()
    return {
        "lexcl": lexcl,
        "ident": ident,
        "rep16flat": rep16flat,
        "rankbase": rankbase,
        "poff": poff,
    }


def _build_program(repeats=1, variant="full"):
    nc = bacc.Bacc(
        "TRN2", target_bir_lowering=False, debug=False, num_devices=N_CORES
    )

    probT = nc.dram_tensor("probT", [SLOTS, B], F32, kind="ExternalInput")
    z_i = nc.dram_tensor("z_i", [B, D], F32, kind="ExternalInput")
    z_j = nc.dram_tensor("z_j", [B, D], F32, kind="ExternalInput")
    wrow = nc.dram_tensor("wrow", [128, NBLK], F32, kind="ExternalInput")
    wfin = nc.dram_tensor("wfin", [1, NBLK], F32, kind="ExternalInput")
    lexcl = nc.dram_tensor("lexcl", [128, 128], F32, kind="ExternalInput")
    ident = nc.dram_tensor("ident", [128, 128], F32, kind="ExternalInput")
    rep16flat = nc.dram_tensor("rep16flat", [1, 2048], F32, kind="ExternalInput")
    rankbase = nc.dram_tensor("rankbase", [128, SLOTS * 8], F32, kind="ExternalInput")
    poff = nc.dram_tensor("poff", [128, 1], F32, kind="ExternalInput")
    outs = [
        nc.dram_tensor(f"partial{r}", [1, 1], F32, kind="ExternalOutput")
        for r in range(repeats)
    ]

    with tile.TileContext(nc) as tc:
        for r in range(repeats):
            _emit(
                nc, tc, probT, z_i, z_j, wrow, wfin, lexcl, ident, rep16flat,
                rankbase, poff, outs[r], rep=r, variant=variant,
            )

    nc.compile()
    return nc


def _emit(nc, tc, probT, z_i, z_j, wrow, wfin, lexcl, ident, rep16flat,
          rankbase, poff, out, rep=0, variant="full"):
    from contextlib import ExitStack

    R = f"r{rep}_"
    ctx = ExitStack()
    with ctx:
        const = ctx.enter_context(tc.tile_pool(name=R + "const", bufs=1))
        main = ctx.enter_context(tc.tile_pool(name=R + "main", bufs=1))
        scr = ctx.enter_context(tc.tile_pool(name=R + "scr", bufs=2))
        escr = ctx.enter_context(tc.tile_pool(name=R + "escr", bufs=4))
        setup_ctx = ExitStack()
        psum_tp = setup_ctx.enter_context(
            tc.tile_pool(name=R + "psum_tp", bufs=2, space="PSUM")
        )
        psum_sm = setup_ctx.enter_context(
            tc.tile_pool(name=R + "psum_sm", bufs=2, space="PSUM")
        )
        dram = ctx.enter_context(tc.tile_pool(name=R + "dram", bufs=1, space="DRAM"))

        # ---- constants -------------------------------------------------
        lexcl_sb = const.tile([128, 128], F32, tag="lexcl")
        ident_sb = const.tile([128, 128], F32, tag="ident")
        rep16f_sb = const.tile([1, 2048], F32, tag="rep16f")
        rankbase_sb = const.tile([128, SLOTS * 8], F32, tag="rankbase")
        poff_sb = const.tile([128, 1], F32, tag="poff")
        wrow_sb = const.tile([128, NBLK], F32, tag="wrow")
        wfin_sb = const.tile([1, NBLK], F32, tag="wfin")
        ones_p = const.tile([128, 1], F32, tag="ones_p")    # column of ones
        ones_r = const.tile([1, 128], F32, tag="ones_r")    # row of ones
        for dst, src in [
            (lexcl_sb, lexcl),
            (ident_sb, ident),
            (rep16f_sb, rep16flat),
            (rankbase_sb, rankbase),
            (poff_sb, poff),
            (wrow_sb, wrow),
            (wfin_sb, wfin),
        ]:
            nc.sync.dma_start(dst[:], src[:])
        nc.vector.memset(ones_p[:], 1.0)
        nc.vector.memset(ones_r[:], 1.0)

        # ---- stage A: prob + thresholds --------------------------------
        prob_sb = main.tile([128, SLOTS, 128], F32, tag="prob")
        # prob_sb[p, c, f] = probT[c, p*128 + f]
        nc.sync.dma_start(
            prob_sb[:], probT.ap().rearrange("c (p f) -> p c f", p=128)
        )
        taus = main.tile([1, 2 * SLOTS], F32, tag="taus")
        for c in range(SLOTS):
            nc.gpsimd.kth_largest(
                taus[0:1, 2 * c : 2 * c + 2],
                prob_sb[:, c, :],
                n_per_lane=128,
                k=K + 2,
                quantile=QUANTILE,
            )
        # broadcast tau (second output = 129th largest) to 128 partitions
        taub_ps = psum_sm.tile([128, SLOTS], F32, tag="sm")
        nc.tensor.matmul(taub_ps[:], ones_r[:], taus[0:1, 1 : 2 * SLOTS : 2])
        taub = main.tile([128, SLOTS], F32, tag="taub_sb")
        nc.vector.tensor_copy(taub[:], taub_ps[:])

        # ---- stage B: index extraction via max8 + local_scatter --------
        vals3 = main.tile([128, SLOTS, 8], F32, tag="vals3")
        idxs3 = main.tile([128, SLOTS, 8], U16, tag="idxs3")
        for c in range(SLOTS):
            nc.vector.max(vals3[:, c, :], prob_sb[:, c, :])
            nc.vector.max_index(idxs3[:, c, :], vals3[:, c, :], prob_sb[:, c, :])
        valid3 = scr.tile([128, SLOTS, 8], F32, tag="valid3")
        nc.vector.tensor_tensor(
            valid3[:],
            vals3[:],
            taub[:].rearrange("p (c o) -> p c o", o=1).to_broadcast([128, SLOTS, 8]),
            op=ALU.is_gt,
        )
        rowcnt = scr.tile([128, SLOTS], F32, tag="rowcnt")
        nc.vector.tensor_reduce(
            rowcnt[:], valid3[:], axis=mybir.AxisListType.X, op=ALU.add
        )
        rowoff_ps = psum_sm.tile([128, SLOTS], F32, tag="sm")
        nc.tensor.matmul(rowoff_ps[:], lexcl_sb[:], rowcnt[:])
        ranks = scr.tile([128, SLOTS, 8], F32, tag="ranks")
        nc.vector.tensor_tensor(
            ranks[:],
            rowoff_ps[:]
            .rearrange("p (c o) -> p c o", o=1)
            .to_broadcast([128, SLOTS, 8]),
            rankbase_sb[:].rearrange("p (c t) -> p c t", t=8),
            op=ALU.add,
        )
        nc.vector.tensor_tensor(ranks[:], ranks[:], valid3[:], op=ALU.mult)
        nc.vector.tensor_scalar_add(ranks[:], ranks[:], -1.0)
        rank16 = main.tile([128, SLOTS * 8], I16, tag="rank16")
        nc.vector.tensor_copy(rank16[:], ranks[:].rearrange("p c t -> p (c t)"))
        # global index = 128 * partition + local index
        gidxf = scr.tile([128, SLOTS * 8], F32, tag="gidxf")
        nc.vector.tensor_copy(gidxf[:], idxs3[:].rearrange("p c t -> p (c t)"))
        nc.vector.tensor_tensor(
            gidxf[:], gidxf[:], poff_sb[:].to_broadcast([128, SLOTS * 8]),
            op=ALU.add,
        )
        cand16 = main.tile([128, SLOTS * 8], I16, tag="cand16")
        nc.vector.tensor_copy(cand16[:], gidxf[:])
        evr16 = main.tile([128, SLOTS * 128], I16, tag="evr16")
        nc.gpsimd.local_scatter(
            evr16[:], cand16[:], rank16[:],
            channels=128, num_elems=SLOTS * 128, num_idxs=SLOTS * 8,
        )
        evrf = scr.tile([128, SLOTS * 128], F32, tag="evrf")
        nc.vector.tensor_copy(evrf[:], evr16[:])
        # compact to a single [1, 896] row (each column has one nonzero)
        allidx_ps = psum_sm.tile([1, SLOTS * 128], F32, tag="smwide")
        nc.tensor.matmul(allidx_ps[:, 0:512], ones_p[:], evrf[:, 0:512])
        nc.tensor.matmul(
            allidx_ps[:, 512 : SLOTS * 128], ones_p[:], evrf[:, 512 : SLOTS * 128]
        )
        allidx = main.tile([1, SLOTS * 128], F32, tag="allidx")
        nc.scalar.copy(allidx[:], allidx_ps[:])
        # redistribute into the 16-partition wrap dma_gather expects
        widx_ps = psum_sm.tile([128, 56], F32, tag="sm")
        av = allidx[:].rearrange("p (s m) -> p m s", m=16)
        for m in range(16):
            nc.tensor.matmul(
                widx_ps[:],
                rep16f_sb[0:1, 128 * m : 128 * (m + 1)],
                av[0:1, m, :],
                start=(m == 0),
                stop=(m == 15),
            )
        idxs_i16 = main.tile([128, 56], I16, tag="idxs")
        nc.vector.tensor_copy(idxs_i16[:], widx_ps[:])

        # ---- stage D: gather selected rows -----------------------------
        gi = main.tile([128, SLOTS, 128], F32, tag="gi")
        gj = main.tile([128, SLOTS, 128], F32, tag="gj")
        for g_sb, z in ((gi, z_i), (gj, z_j)):
            nc.gpsimd.dma_gather(
                g_sb[:],
                z.ap(),
                idxs_i16[:],
                num_idxs=SLOTS * 128,
                num_idxs_reg=SLOTS * 128,
                elem_size=D,
            )

        # ---- stage E: normalize + transpose into bf16 table ------------
        # 1/|z| = exp(-0.5 * ln(|z|^2)) keeps every activation in the
        # natural_log_exp_and_others table (no Sqrt -> no table reloads).
        sqs = main.tile([128, NBLK], F32, tag="sqs")
        for b in range(NBLK):
            s, h = b // 2, b % 2
            src = (gi if h == 0 else gj)[:, s, :]
            trash = scr.tile([128, 128], F32, tag="trash")
            nc.scalar.activation(
                trash[:], src, AF.Square, accum_out=sqs[:, b : b + 1]
            )
        lnv = scr.tile([128, NBLK], F32, tag="lnv")
        nc.scalar.activation(lnv[:], sqs[:], AF.Ln)
        rnw = main.tile([128, NBLK], F32, tag="rnw")
        nc.scalar.activation(rnw[:], lnv[:], AF.Exp, scale=-0.5)
        nc.vector.tensor_tensor(rnw[:], rnw[:], wrow_sb[:], op=ALU.mult)

        agin = [
            dram.tile([128, CH_SZ[k]], BF16, name=f"agin{k}") for k in range(4)
        ]
        agout = [
            dram.tile(
                [N_CORES * 128, CH_SZ[k]], BF16, addr_space="Shared",
                name=f"agout{k}",
            )
            for k in range(4)
        ]
        flatT = main.tile([128, TBL], BF16, tag="flatT")
        ag_fired = 0
        for b in range(NBLK):
            s, h = b // 2, b % 2
            src = (gi if h == 0 else gj)[:, s, :]
            diag = scr.tile([128, 128], F32, tag="diag")
            nc.vector.tensor_tensor(
                diag[:], ident_sb[:], rnw[:, b : b + 1].to_broadcast([128, 128]),
                op=ALU.mult,
            )
            tp_ps = psum_tp.tile([128, 128], F32, tag="tp")
            nc.tensor.matmul(tp_ps[:], src, diag[:])
            nc.scalar.copy(
                flatT[:, 256 * s + 128 * h : 256 * s + 128 * h + 128], tp_ps[:]
            )
            # fire AllGather chunks as soon as their columns are built
            while ag_fired < 4 and b == CH_DONE_B[ag_fired]:
                k = ag_fired
                if "noag" not in variant:
                    nc.sync.dma_start(
                        agin[k][:], flatT[:, CH_OFF[k] : CH_OFF[k] + CH_SZ[k]]
                    )
                    nc.gpsimd.collective_compute(
                        "AllGather",
                        ALU.bypass,
                        replica_groups=[list(range(N_CORES))],
                        ins=[agin[k].opt()],
                        outs=[agout[k].opt()],
                    )
                ag_fired += 1

        # ---- stage F: rotated reload of peer chunks --------------------
        if "noag" not in variant:
            pid = nc.partition_id()
            for k in range(4):
                for j in range(1, N_CORES):
                    rj = (pid + j) & 7
                    dst = GOFF[k] + (j - 1) * CH_SZ[k]
                    nc.sync.dma_start(
                        flatT[:, dst : dst + CH_SZ[k]],
                        agout[k][bass.ds(rj * 128, 128), :],
                    )
        else:
            # fill peer columns locally (wrong numerics, same sweep timing)
            for k in range(4):
                for j in range(1, N_CORES):
                    dst = GOFF[k] + (j - 1) * CH_SZ[k]
                    nc.sync.dma_start(
                        flatT[:, dst : dst + CH_SZ[k]],
                        flatT[:, CH_OFF[k] : CH_OFF[k] + CH_SZ[k]],
                    )

        # ---- stage G: sim sweep (supertile-major) ----------------------
        setup_ctx.close()
        psum_sim = ctx.enter_context(
            tc.tile_pool(name=R + "psum_sim", bufs=2, space="PSUM")
        )
        if "nosweep" in variant:
            nos = main.tile([1, 1], F32, tag="nos")
            nc.vector.tensor_reduce(
                nos[:], flatT[0:1, :], axis=mybir.AxisListType.X, op=ALU.add
            )
            nc.sync.dma_start(out[:], nos[:])
            return
        partials = main.tile([128, NBLK, NST], F32, tag="partials")
        own_t = main.tile([128, NBLK], F32, tag="own_t")
        pos_t = main.tile([128, NBLK], F32, tag="pos_t")
        for st in range(NST):
            for b in range(NBLK):
                s, h = b // 2, b % 2
                lhsT = flatT[:, 256 * s + 128 * h : 256 * s + 128 * h + 128]
                sim_ps = psum_sim.tile([128, 2048], F32, tag="sim")
                for q0, qs in ((0, 512), (512, 512), (1024, 512), (1536, 256)):
                    nc.tensor.matmul(
                        sim_ps[:, q0 : q0 + qs],
                        lhsT,
                        flatT[:, ST * st + q0 : ST * st + q0 + qs],
                    )
                e_sb = escr.tile([128, ST], BF16, tag="e")
                nc.scalar.activation(
                    e_sb[:],
                    sim_ps[:, 0:ST],
                    AF.Exp,
                    scale=1.0 / TEMP,
                    accum_out=partials[:, b, st : st + 1],
                )
                if st == 0:
                    off = 256 * s
                    nc.vector.tensor_reduce(
                        pos_t[:, b : b + 1],
                        e_sb[:, off : off + 128],
                        axis=mybir.AxisListType.X,
                        op=ALU.add,
                    )
                    nc.vector.tensor_reduce(
                        own_t[:, b : b + 1],
                        e_sb[:, off : off + 256],
                        axis=mybir.AxisListType.X,
                        op=ALU.add,
                    )

        # ---- stage H: reduce to one scalar -----------------------------
        totals = main.tile([128, NBLK], F32, tag="totals")
        nc.vector.tensor_reduce(
            totals[:], partials[:], axis=mybir.AxisListType.X, op=ALU.add
        )
        neg = scr.tile([128, NBLK], F32, tag="neg")
        nc.vector.scalar_tensor_tensor(
            neg[:], totals[:], float(-N_DUMMY_COLS), own_t[:],
            op0=ALU.add, op1=ALU.subtract,
        )
        lnn = scr.tile([128, NBLK], F32, tag="lnn")
        lnp = scr.tile([128, NBLK], F32, tag="lnp")
        nc.scalar.activation(lnn[:], neg[:], AF.Ln)
        nc.scalar.activation(lnp[:], pos_t[:], AF.Ln)
        lrows = main.tile([128, NBLK], F32, tag="lrows")
        nc.vector.tensor_sub(lrows[:], lnn[:], lnp[:])
        fin_ps = psum_sim.tile([1, NBLK], F32, tag="sim")
        nc.tensor.matmul(fin_ps[:], ones_p[:], lrows[:])
        fin_sb = main.tile([1, NBLK], F32, tag="fin_sb")
        nc.vector.tensor_tensor(fin_sb[:], fin_ps[:], wfin_sb[:], op=ALU.mult)
        out_sb = main.tile([1, 1], F32, tag="out_sb")
        nc.vector.tensor_reduce(
            out_sb[:], fin_sb[:], axis=mybir.AxisListType.X, op=ALU.add
        )
        nc.vector.tensor_scalar_mul(out_sb[:], out_sb[:], 1.0 / (2 * K * C))
        nc.sync.dma_start(out[:], out_sb[:])


def _per_core_inputs(prob, z_i, z_j):
    consts = _host_constants()
    maps = []
    for k in range(N_CORES):
        ncl = CCNT[k]
        cols = list(range(CBASE[k], CBASE[k] + ncl))
        cols = cols + [CBASE[k]] * (SLOTS - ncl)  # dummy slots reuse first col
        pT = np.ascontiguousarray(prob[:, cols].T)  # [SLOTS, B]
        w = np.array([1.0] * ncl + [0.0] * (SLOTS - ncl), dtype=np.float32)
        wrow = np.broadcast_to(
            np.repeat(w, 2)[None, :], (128, NBLK)
        ).copy()  # [128, 14]
        wfin = np.repeat(w, 2)[None, :].astype(np.float32).copy()  # [1, 14]
        m = {
            "probT": pT,
            "z_i": z_i,
            "z_j": z_j,
            "wrow": wrow,
            "wfin": wfin,
        }
        m.update(consts)
        maps.append(m)
    return maps


def kernel(prob, z_i, z_j):
    if "nc" not in _CACHE:
        _CACHE["nc"] = _build_program()
    nc = _CACHE["nc"]
    in_maps = _per_core_inputs(
        np.asarray(prob, dtype=np.float32),
        np.ascontiguousarray(z_i, dtype=np.float32),
        np.ascontiguousarray(z_j, dtype=np.float32),
    )
    res = run_bass_kernel_spmd(nc, in_maps, list(range(N_CORES)))
    total = np.float32(0.0)
    for r in res.results:
        total += r["partial0"][0, 0]
    return np.asarray(total, dtype=np.float32)


# revision 11
# speedup vs baseline: 1.0427x; 1.0104x over previous
"""Cluster-based contrastive loss on 8 Trainium2 NeuronCores.

Strategy: shard the C=50 cluster axis across 8 cores (7 slots/core, padded
to 56 global slots; 6 dummy slots carry weight 0).  Each core:
  - computes exact top-128 thresholds for its clusters (gpsimd kth_largest
    returns the 129th-largest value exactly),
  - finds per-partition top-8 values+indices (DVE max/max_index), ranks the
    survivors with a prefix-sum matmul and compacts them with a gpsimd
    local_scatter (one-hot matmul eliminated),
  - gathers the selected z_i / z_j rows from HBM (dma_gather),
  - normalizes (exp(-0.5*ln(|z|^2)) so the whole kernel uses one ACT
    function table) + transposes into a bf16 [128, D->partition] column
    table via a PE matmul against diag(w / ||z||),
  - AllGathers the table in 4 column-chunks (fired as soon as each chunk
    is built) so the collective overlaps the sim sweep,
  - sweeps flatT.T @ flatT supertile-major (1792 cols per supertile) with
    ACT exp (accum_out gives row sums free), subtracts the own-cluster
    block, and reduces log(neg) - log(pos) to one scalar.
The host sums the 8 per-core partial scalars.
"""

import sys

sys.path.insert(0, "/opt/trn_rl_repo")

import numpy as np

import concourse.bacc as bacc
import concourse.bass as bass
import concourse.mybir as mybir
from concourse import tile
from concourse.bass_utils import run_bass_kernel_spmd

F32 = mybir.dt.float32
BF16 = mybir.dt.bfloat16
I16 = mybir.dt.int16
U16 = mybir.dt.uint16
AF = mybir.ActivationFunctionType
ALU = mybir.AluOpType

B = 16384
D = 128
C = 50
K = 128
TEMP = 0.5
N_CORES = 8
SLOTS = 7                      # cluster slots per core
GSLOTS = N_CORES * SLOTS       # 56 global slots
TBL = GSLOTS * 2 * K           # 14336 columns in the padded table
LOCAL = SLOTS * 2 * K          # 1792 columns contributed per core
NBLK = SLOTS * 2               # 14 row blocks per core
N_DUMMY_COLS = (GSLOTS - C) * 2 * K   # 1536 zero columns in the table
# core k owns clusters [CBASE[k], CBASE[k] + CCNT[k])
CCNT = [7, 7, 6, 6, 6, 6, 6, 6]
CBASE = [0, 7, 14, 20, 26, 32, 38, 44]
QUANTILE = 1.0 - 127.5 / (B - 1)
# AllGather column chunks of the local table (fired as stage E completes
# each range); group k of the sweep table holds all 7 peers' chunk k.
CH_OFF = [0, 512, 1024, 1536]
CH_SZ = [512, 512, 512, 256]
CH_DONE_B = [3, 7, 11, 13]     # last stage-E block (2s+h) filling each chunk
GOFF = [1792, 5376, 8960, 12544]
ST = 1792                      # sweep supertile width; TBL = 8 * ST
NST = TBL // ST

_CACHE = {}


def _host_constants():
    lexcl = (np.arange(128)[:, None] < np.arange(128)[None, :]).astype(np.float32)
    ident = np.eye(128, dtype=np.float32)
    # rep16flat[0, 128*m + j] = (j % 16 == m): outer-product rows used to
    # redistribute the compacted [1, 896] index row into 16-partition wrap.
    rep16flat = np.zeros((1, 16 * 128), dtype=np.float32)
    for m in range(16):
        rep16flat[0, 128 * m + np.arange(m, 128, 16)] = 1.0
    # rankbase[p, 8c+t] = 128*c + t + 1  (cluster-global scatter slot, +1 so
    # invalid lanes become -1 after (x * valid) - 1)
    rb = (128 * np.arange(SLOTS)[:, None] + np.arange(8)[None, :] + 1).astype(
        np.float32
    )
    rankbase = np.broadcast_to(rb.reshape(1, SLOTS * 8), (128, SLOTS * 8)).copy()
    poff = (128.0 * np.arange(128, dtype=np.float32))[:, None].copy()
    return {
        "lexcl": lexcl,
        "ident": ident,
        "rep16flat": rep16flat,
        "rankbase": rankbase,
        "poff": poff,
    }


def _build_program(repeats=1, variant="full"):
    nc = bacc.Bacc(
        "TRN2", target_bir_lowering=False, debug=False, num_devices=N_CORES
    )

    probT = nc.dram_tensor("probT", [SLOTS, B], F32, kind="ExternalInput")
    z_i = nc.dram_tensor("z_i", [B, D], F32, kind="ExternalInput")
    z_j = nc.dram_tensor("z_j", [B, D], F32, kind="ExternalInput")
    wrow = nc.dram_tensor("wrow", [128, NBLK], F32, kind="ExternalInput")
    wfin = nc.dram_tensor("wfin", [1, NBLK], F32, kind="ExternalInput")
    lexcl = nc.dram_tensor("lexcl", [128, 128], F32, kind="ExternalInput")
    ident = nc.dram_tensor("ident", [128, 128], F32, kind="ExternalInput")
    rep16flat = nc.dram_tensor("rep16flat", [1, 2048], F32, kind="ExternalInput")
    rankbase = nc.dram_tensor("rankbase", [128, SLOTS * 8], F32, kind="ExternalInput")
    poff = nc.dram_tensor("poff", [128, 1], F32, kind="ExternalInput")
    outs = [
        nc.dram_tensor(f"partial{r}", [1, 1], F32, kind="ExternalOutput")
        for r in range(repeats)
    ]

    with tile.TileContext(nc) as tc:
        for r in range(repeats):
            _emit(
                nc, tc, probT, z_i, z_j, wrow, wfin, lexcl, ident, rep16flat,
                rankbase, poff, outs[r], rep=r, variant=variant,
            )

    nc.compile()
    return nc


def _emit(nc, tc, probT, z_i, z_j, wrow, wfin, lexcl, ident, rep16flat,
          rankbase, poff, out, rep=0, variant="full"):
    from contextlib import ExitStack

    R = f"r{rep}_"
    ctx = ExitStack()
    with ctx:
        const = ctx.enter_context(tc.tile_pool(name=R + "const", bufs=1))
        main = ctx.enter_context(tc.tile_pool(name=R + "main", bufs=1))
        scr = ctx.enter_context(tc.tile_pool(name=R + "scr", bufs=2))
        escr = ctx.enter_context(tc.tile_pool(name=R + "escr", bufs=4))
        setup_ctx = ExitStack()
        psum_tp = setup_ctx.enter_context(
            tc.tile_pool(name=R + "psum_tp", bufs=2, space="PSUM")
        )
        psum_sm = setup_ctx.enter_context(
            tc.tile_pool(name=R + "psum_sm", bufs=2, space="PSUM")
        )
        dram = ctx.enter_context(tc.tile_pool(name=R + "dram", bufs=1, space="DRAM"))

        # ---- constants -------------------------------------------------
        lexcl_sb = const.tile([128, 128], F32, tag="lexcl")
        ident_sb = const.tile([128, 128], F32, tag="ident")
        rep16f_sb = const.tile([1, 2048], F32, tag="rep16f")
        rankbase_sb = const.tile([128, SLOTS * 8], F32, tag="rankbase")
        poff_sb = const.tile([128, 1], F32, tag="poff")
        wrow_sb = const.tile([128, NBLK], F32, tag="wrow")
        wfin_sb = const.tile([1, NBLK], F32, tag="wfin")
        ones_p = const.tile([128, 1], F32, tag="ones_p")    # column of ones
        ones_r = const.tile([1, 128], F32, tag="ones_r")    # row of ones
        for dst, src in [
            (lexcl_sb, lexcl),
            (ident_sb, ident),
            (rep16f_sb, rep16flat),
            (rankbase_sb, rankbase),
            (poff_sb, poff),
            (wrow_sb, wrow),
            (wfin_sb, wfin),
        ]:
            nc.sync.dma_start(dst[:], src[:])
        nc.vector.memset(ones_p[:], 1.0)
        nc.vector.memset(ones_r[:], 1.0)

        # ---- stage A: prob + thresholds --------------------------------
        prob_sb = main.tile([128, SLOTS, 128], F32, tag="prob")
        # prob_sb[p, c, f] = probT[c, p*128 + f]
        nc.sync.dma_start(
            prob_sb[:], probT.ap().rearrange("c (p f) -> p c f", p=128)
        )
        if "mini" in variant:
            mini_ps = psum_sm.tile([128, 128], F32, tag="sm128")
            nc.tensor.matmul(mini_ps[:], lexcl_sb[:], prob_sb[:, 0, :])
            mini = main.tile([1, 1], F32, tag="mini")
            nc.vector.tensor_reduce(
                mini[:], mini_ps[0:1, :], axis=mybir.AxisListType.X, op=ALU.add
            )
            nc.sync.dma_start(out[:], mini[:])
            setup_ctx.close()
            return
        taus = main.tile([1, 2 * SLOTS], F32, tag="taus")
        for c in range(SLOTS):
            nc.gpsimd.kth_largest(
                taus[0:1, 2 * c : 2 * c + 2],
                prob_sb[:, c, :],
                n_per_lane=128,
                k=K + 2,
                quantile=QUANTILE,
            )
        # broadcast tau (second output = 129th largest) to 128 partitions
        taub_ps = psum_sm.tile([128, SLOTS], F32, tag="sm")
        nc.tensor.matmul(taub_ps[:], ones_r[:], taus[0:1, 1 : 2 * SLOTS : 2])
        taub = main.tile([128, SLOTS], F32, tag="taub_sb")
        nc.vector.tensor_copy(taub[:], taub_ps[:])

        # ---- stage B: index extraction via max8 + local_scatter --------
        vals3 = main.tile([128, SLOTS, 8], F32, tag="vals3")
        idxs3 = main.tile([128, SLOTS, 8], U16, tag="idxs3")
        for c in range(SLOTS):
            nc.vector.max(vals3[:, c, :], prob_sb[:, c, :])
            nc.vector.max_index(idxs3[:, c, :], vals3[:, c, :], prob_sb[:, c, :])
        valid3 = scr.tile([128, SLOTS, 8], F32, tag="valid3")
        nc.vector.tensor_tensor(
            valid3[:],
            vals3[:],
            taub[:].rearrange("p (c o) -> p c o", o=1).to_broadcast([128, SLOTS, 8]),
            op=ALU.is_gt,
        )
        rowcnt = scr.tile([128, SLOTS], F32, tag="rowcnt")
        nc.vector.tensor_reduce(
            rowcnt[:], valid3[:], axis=mybir.AxisListType.X, op=ALU.add
        )
        rowoff_ps = psum_sm.tile([128, SLOTS], F32, tag="sm")
        nc.tensor.matmul(rowoff_ps[:], lexcl_sb[:], rowcnt[:])
        ranks = scr.tile([128, SLOTS, 8], F32, tag="ranks")
        nc.vector.tensor_tensor(
            ranks[:],
            rowoff_ps[:]
            .rearrange("p (c o) -> p c o", o=1)
            .to_broadcast([128, SLOTS, 8]),
            rankbase_sb[:].rearrange("p (c t) -> p c t", t=8),
            op=ALU.add,
        )
        nc.vector.tensor_tensor(ranks[:], ranks[:], valid3[:], op=ALU.mult)
        nc.vector.tensor_scalar_add(ranks[:], ranks[:], -1.0)
        rank16 = main.tile([128, SLOTS * 8], I16, tag="rank16")
        nc.vector.tensor_copy(rank16[:], ranks[:].rearrange("p c t -> p (c t)"))
        # global index = 128 * partition + local index
        gidxf = scr.tile([128, SLOTS * 8], F32, tag="gidxf")
        nc.vector.tensor_copy(gidxf[:], idxs3[:].rearrange("p c t -> p (c t)"))
        nc.vector.tensor_tensor(
            gidxf[:], gidxf[:], poff_sb[:].to_broadcast([128, SLOTS * 8]),
            op=ALU.add,
        )
        cand16 = main.tile([128, SLOTS * 8], I16, tag="cand16")
        nc.vector.tensor_copy(cand16[:], gidxf[:])
        evr16 = main.tile([128, SLOTS * 128], I16, tag="evr16")
        nc.gpsimd.local_scatter(
            evr16[:], cand16[:], rank16[:],
            channels=128, num_elems=SLOTS * 128, num_idxs=SLOTS * 8,
        )
        evrf = scr.tile([128, SLOTS * 128], F32, tag="evrf")
        nc.vector.tensor_copy(evrf[:], evr16[:])
        # compact to a single [1, 896] row (each column has one nonzero)
        allidx_ps = psum_sm.tile([1, SLOTS * 128], F32, tag="smwide")
        nc.tensor.matmul(allidx_ps[:, 0:512], ones_p[:], evrf[:, 0:512])
        nc.tensor.matmul(
            allidx_ps[:, 512 : SLOTS * 128], ones_p[:], evrf[:, 512 : SLOTS * 128]
        )
        allidx = main.tile([1, SLOTS * 128], F32, tag="allidx")
        nc.scalar.copy(allidx[:], allidx_ps[:])
        # redistribute into the 16-partition wrap dma_gather expects
        widx_ps = psum_sm.tile([128, 56], F32, tag="sm")
        av = allidx[:].rearrange("p (s m) -> p m s", m=16)
        for m in range(16):
            nc.tensor.matmul(
                widx_ps[:],
                rep16f_sb[0:1, 128 * m : 128 * (m + 1)],
                av[0:1, m, :],
                start=(m == 0),
                stop=(m == 15),
            )
        idxs_i16 = main.tile([128, 56], I16, tag="idxs")
        nc.vector.tensor_copy(idxs_i16[:], widx_ps[:])

        # ---- stage D: gather selected rows -----------------------------
        gi = main.tile([128, SLOTS, 128], F32, tag="gi")
        gj = main.tile([128, SLOTS, 128], F32, tag="gj")
        for g_sb, z in ((gi, z_i), (gj, z_j)):
            nc.gpsimd.dma_gather(
                g_sb[:],
                z.ap(),
                idxs_i16[:],
                num_idxs=SLOTS * 128,
                num_idxs_reg=SLOTS * 128,
                elem_size=D,
            )

        # ---- stage E: normalize + transpose into bf16 table ------------
        # 1/|z| = exp(-0.5 * ln(|z|^2)) keeps every activation in the
        # natural_log_exp_and_others table (no Sqrt -> no table reloads).
        sqs = main.tile([128, NBLK], F32, tag="sqs")
        for b in range(NBLK):
            s, h = b // 2, b % 2
            src = (gi if h == 0 else gj)[:, s, :]
            trash = scr.tile([128, 128], F32, tag="trash")
            nc.scalar.activation(
                trash[:], src, AF.Square, accum_out=sqs[:, b : b + 1]
            )
        lnv = scr.tile([128, NBLK], F32, tag="lnv")
        nc.scalar.activation(lnv[:], sqs[:], AF.Ln)
        rnw = main.tile([128, NBLK], F32, tag="rnw")
        nc.scalar.activation(rnw[:], lnv[:], AF.Exp, scale=-0.5)
        nc.vector.tensor_tensor(rnw[:], rnw[:], wrow_sb[:], op=ALU.mult)

        agin = [
            dram.tile([128, CH_SZ[k]], BF16, name=f"agin{k}") for k in range(4)
        ]
        agout = [
            dram.tile(
                [N_CORES * 128, CH_SZ[k]], BF16, addr_space="Shared",
                name=f"agout{k}",
            )
            for k in range(4)
        ]
        flatT = main.tile([128, TBL], BF16, tag="flatT")
        ag_fired = 0
        for b in range(NBLK):
            s, h = b // 2, b % 2
            src = (gi if h == 0 else gj)[:, s, :]
            diag = scr.tile([128, 128], F32, tag="diag")
            nc.vector.tensor_tensor(
                diag[:], ident_sb[:], rnw[:, b : b + 1].to_broadcast([128, 128]),
                op=ALU.mult,
            )
            tp_ps = psum_tp.tile([128, 128], F32, tag="tp")
            nc.tensor.matmul(tp_ps[:], src, diag[:])
            nc.scalar.copy(
                flatT[:, 256 * s + 128 * h : 256 * s + 128 * h + 128], tp_ps[:]
            )
            # fire AllGather chunks as soon as their columns are built
            while ag_fired < 4 and b == CH_DONE_B[ag_fired]:
                k = ag_fired
                if "noag" not in variant:
                    nc.sync.dma_start(
                        agin[k][:], flatT[:, CH_OFF[k] : CH_OFF[k] + CH_SZ[k]]
                    )
                    nc.gpsimd.collective_compute(
                        "AllGather",
                        ALU.bypass,
                        replica_groups=[list(range(N_CORES))],
                        ins=[agin[k].opt()],
                        outs=[agout[k].opt()],
                    )
                ag_fired += 1

        # ---- stage F: rotated reload of peer chunks --------------------
        if "noag" not in variant:
            pid = nc.partition_id()
            for k in range(4):
                for j in range(1, N_CORES):
                    rj = (pid + j) & 7
                    dst = GOFF[k] + (j - 1) * CH_SZ[k]
                    nc.sync.dma_start(
                        flatT[:, dst : dst + CH_SZ[k]],
                        agout[k][bass.ds(rj * 128, 128), :],
                    )
        else:
            # fill peer columns locally (wrong numerics, same sweep timing)
            for k in range(4):
                for j in range(1, N_CORES):
                    dst = GOFF[k] + (j - 1) * CH_SZ[k]
                    nc.sync.dma_start(
                        flatT[:, dst : dst + CH_SZ[k]],
                        flatT[:, CH_OFF[k] : CH_OFF[k] + CH_SZ[k]],
                    )

        # ---- stage G: sim sweep (supertile-major) ----------------------
        setup_ctx.close()
        psum_sim = ctx.enter_context(
            tc.tile_pool(name=R + "psum_sim", bufs=2, space="PSUM")
        )
        if "nosweep" in variant:
            nos = main.tile([1, 1], F32, tag="nos")
            nc.vector.tensor_reduce(
                nos[:], flatT[0:1, :], axis=mybir.AxisListType.X, op=ALU.add
            )
            nc.sync.dma_start(out[:], nos[:])
            return
        partials = main.tile([128, NBLK, NST], F32, tag="partials")
        own_t = main.tile([128, NBLK], F32, tag="own_t")
        pos_t = main.tile([128, NBLK], F32, tag="pos_t")
        for st in range(NST):
            for b in range(NBLK):
                s, h = b // 2, b % 2
                lhsT = flatT[:, 256 * s + 128 * h : 256 * s + 128 * h + 128]
                sim_ps = psum_sim.tile([128, 2048], F32, tag="sim")
                for q0, qs in ((0, 512), (512, 512), (1024, 512), (1536, 256)):
                    nc.tensor.matmul(
                        sim_ps[:, q0 : q0 + qs],
                        lhsT,
                        flatT[:, ST * st + q0 : ST * st + q0 + qs],
                    )
                e_sb = escr.tile([128, ST], BF16, tag="e")
                nc.scalar.activation(
                    e_sb[:],
                    sim_ps[:, 0:ST],
                    AF.Exp,
                    scale=1.0 / TEMP,
                    accum_out=partials[:, b, st : st + 1],
                )
                if st == 0:
                    off = 256 * s
                    nc.vector.tensor_reduce(
                        pos_t[:, b : b + 1],
                        e_sb[:, off : off + 128],
                        axis=mybir.AxisListType.X,
                        op=ALU.add,
                    )
                    nc.vector.tensor_reduce(
                        own_t[:, b : b + 1],
                        e_sb[:, off : off + 256],
                        axis=mybir.AxisListType.X,
                        op=ALU.add,
                    )

        # ---- stage H: reduce to one scalar -----------------------------
        totals = main.tile([128, NBLK], F32, tag="totals")
        nc.vector.tensor_reduce(
            totals[:], partials[:], axis=mybir.AxisListType.X, op=ALU.add
        )
        neg = scr.tile([128, NBLK], F32, tag="neg")
        nc.vector.scalar_tensor_tensor(
            neg[:], totals[:], float(-N_DUMMY_COLS), own_t[:],
            op0=ALU.add, op1=ALU.subtract,
        )
        lnn = scr.tile([128, NBLK], F32, tag="lnn")
        lnp = scr.tile([128, NBLK], F32, tag="lnp")
        nc.scalar.activation(lnn[:], neg[:], AF.Ln)
        nc.scalar.activation(lnp[:], pos_t[:], AF.Ln)
        lrows = main.tile([128, NBLK], F32, tag="lrows")
        nc.vector.tensor_sub(lrows[:], lnn[:], lnp[:])
        fin_ps = psum_sim.tile([1, NBLK], F32, tag="sim")
        nc.tensor.matmul(fin_ps[:], ones_p[:], lrows[:])
        fin_sb = main.tile([1, NBLK], F32, tag="fin_sb")
        nc.vector.tensor_tensor(fin_sb[:], fin_ps[:], wfin_sb[:], op=ALU.mult)
        out_sb = main.tile([1, 1], F32, tag="out_sb")
        nc.vector.tensor_reduce(
            out_sb[:], fin_sb[:], axis=mybir.AxisListType.X, op=ALU.add
        )
        nc.vector.tensor_scalar_mul(out_sb[:], out_sb[:], 1.0 / (2 * K * C))
        nc.sync.dma_start(out[:], out_sb[:])


def _per_core_inputs(prob, z_i, z_j):
    consts = _host_constants()
    maps = []
    for k in range(N_CORES):
        ncl = CCNT[k]
        cols = list(range(CBASE[k], CBASE[k] + ncl))
        cols = cols + [CBASE[k]] * (SLOTS - ncl)  # dummy slots reuse first col
        pT = np.ascontiguousarray(prob[:, cols].T)  # [SLOTS, B]
        w = np.array([1.0] * ncl + [0.0] * (SLOTS - ncl), dtype=np.float32)
        wrow = np.broadcast_to(
            np.repeat(w, 2)[None, :], (128, NBLK)
        ).copy()  # [128, 14]
        wfin = np.repeat(w, 2)[None, :].astype(np.float32).copy()  # [1, 14]
        m = {
            "probT": pT,
            "z_i": z_i,
            "z_j": z_j,
            "wrow": wrow,
            "wfin": wfin,
        }
        m.update(consts)
        maps.append(m)
    return maps


def kernel(prob, z_i, z_j):
    if "nc" not in _CACHE:
        _CACHE["nc"] = _build_program()
    nc = _CACHE["nc"]
    in_maps = _per_core_inputs(
        np.asarray(prob, dtype=np.float32),
        np.ascontiguousarray(z_i, dtype=np.float32),
        np.ascontiguousarray(z_j, dtype=np.float32),
    )
    res = run_bass_kernel_spmd(nc, in_maps, list(range(N_CORES)))
    total = np.float32(0.0)
    for r in res.results:
        total += r["partial0"][0, 0]
    return np.asarray(total, dtype=np.float32)


# revision 12
# speedup vs baseline: 1.0956x; 1.0508x over previous
"""Cluster-based contrastive loss on 8 Trainium2 NeuronCores.

Strategy: shard the C=50 cluster axis across 8 cores (7 slots/core, padded
to 56 global slots; 6 dummy slots carry weight 0).  Each core:
  - computes exact top-128 thresholds for its clusters (gpsimd kth_largest
    returns the 129th-largest value exactly),
  - finds per-partition top-8 values+indices (DVE max/max_index), ranks the
    survivors with a prefix-sum matmul and compacts them with a gpsimd
    local_scatter (one-hot matmul eliminated),
  - gathers the selected z_i / z_j rows from HBM (dma_gather),
  - normalizes (exp(-0.5*ln(|z|^2)) so the whole kernel uses one ACT
    function table) + transposes into a bf16 [128, D->partition] column
    table via a PE matmul against diag(w / ||z||),
  - AllGathers the table in 4 column-chunks (fired as soon as each chunk
    is built) so the collective overlaps the sim sweep,
  - sweeps flatT.T @ flatT supertile-major (1792 cols per supertile) with
    ACT exp (accum_out gives row sums free), subtracts the own-cluster
    block, and reduces log(neg) - log(pos) to one scalar.
The host sums the 8 per-core partial scalars.
"""

import sys

sys.path.insert(0, "/opt/trn_rl_repo")

import numpy as np

import concourse.bacc as bacc
import concourse.bass as bass
import concourse.mybir as mybir
from concourse import tile
from concourse.bass_utils import run_bass_kernel_spmd

F32 = mybir.dt.float32
BF16 = mybir.dt.bfloat16
I16 = mybir.dt.int16
U16 = mybir.dt.uint16
AF = mybir.ActivationFunctionType
ALU = mybir.AluOpType

B = 16384
D = 128
C = 50
K = 128
TEMP = 0.5
N_CORES = 8
SLOTS = 7                      # cluster slots per core
GSLOTS = N_CORES * SLOTS       # 56 global slots
TBL = GSLOTS * 2 * K           # 14336 columns in the padded table
LOCAL = SLOTS * 2 * K          # 1792 columns contributed per core
NBLK = SLOTS * 2               # 14 row blocks per core
N_DUMMY_COLS = (GSLOTS - C) * 2 * K   # 1536 zero columns in the table
# core k owns clusters [CBASE[k], CBASE[k] + CCNT[k])
CCNT = [7, 7, 6, 6, 6, 6, 6, 6]
CBASE = [0, 7, 14, 20, 26, 32, 38, 44]
QUANTILE = 1.0 - 127.5 / (B - 1)
# AllGather column chunks of the local table (fired as stage E completes
# each range); group k of the sweep table holds all 7 peers' chunk k.
CH_OFF = [0, 512, 1024, 1536]
CH_SZ = [512, 512, 512, 256]
CH_DONE_B = [3, 7, 11, 13]     # last stage-E block (2s+h) filling each chunk
GOFF = [1792, 5376, 8960, 12544]
ST = 1792                      # sweep supertile width; TBL = 8 * ST
NST = TBL // ST

_CACHE = {}


def _host_constants():
    lexcl = (np.arange(128)[:, None] < np.arange(128)[None, :]).astype(np.float32)
    ident = np.eye(128, dtype=np.float32)
    # rep16flat[0, 128*m + j] = (j % 16 == m): outer-product rows used to
    # redistribute the compacted [1, 896] index row into 16-partition wrap.
    rep16flat = np.zeros((1, 16 * 128), dtype=np.float32)
    for m in range(16):
        rep16flat[0, 128 * m + np.arange(m, 128, 16)] = 1.0
    # rankbase[p, 8c+t] = 128*c + t + 1  (cluster-global scatter slot, +1 so
    # invalid lanes become -1 after (x * valid) - 1)
    rb = (128 * np.arange(SLOTS)[:, None] + np.arange(8)[None, :] + 1).astype(
        np.float32
    )
    rankbase = np.broadcast_to(rb.reshape(1, SLOTS * 8), (128, SLOTS * 8)).copy()
    poff = (128.0 * np.arange(128, dtype=np.float32))[:, None].copy()
    return {
        "lexcl": lexcl,
        "ident": ident,
        "rep16flat": rep16flat,
        "rankbase": rankbase,
        "poff": poff,
    }


def _build_program(repeats=1, variant="full"):
    nc = bacc.Bacc(
        "TRN2", target_bir_lowering=False, debug=False, num_devices=N_CORES
    )

    probT = nc.dram_tensor("probT", [SLOTS, B], F32, kind="ExternalInput")
    z_i = nc.dram_tensor("z_i", [B, D], F32, kind="ExternalInput")
    z_j = nc.dram_tensor("z_j", [B, D], F32, kind="ExternalInput")
    wrow = nc.dram_tensor("wrow", [128, NBLK], F32, kind="ExternalInput")
    wfin = nc.dram_tensor("wfin", [1, NBLK], F32, kind="ExternalInput")
    lexcl = nc.dram_tensor("lexcl", [128, 128], F32, kind="ExternalInput")
    ident = nc.dram_tensor("ident", [128, 128], F32, kind="ExternalInput")
    rep16flat = nc.dram_tensor("rep16flat", [1, 2048], F32, kind="ExternalInput")
    rankbase = nc.dram_tensor("rankbase", [128, SLOTS * 8], F32, kind="ExternalInput")
    poff = nc.dram_tensor("poff", [128, 1], F32, kind="ExternalInput")
    outs = [
        nc.dram_tensor(f"partial{r}", [1, 1], F32, kind="ExternalOutput")
        for r in range(repeats)
    ]

    with tile.TileContext(nc) as tc:
        for r in range(repeats):
            _emit(
                nc, tc, probT, z_i, z_j, wrow, wfin, lexcl, ident, rep16flat,
                rankbase, poff, outs[r], rep=r, variant=variant,
            )

    nc.compile()
    return nc


def _emit(nc, tc, probT, z_i, z_j, wrow, wfin, lexcl, ident, rep16flat,
          rankbase, poff, out, rep=0, variant="full"):
    from contextlib import ExitStack

    R = f"r{rep}_"
    ctx = ExitStack()
    with ctx:
        const = ctx.enter_context(tc.tile_pool(name=R + "const", bufs=1))
        main = ctx.enter_context(tc.tile_pool(name=R + "main", bufs=1))
        scr = ctx.enter_context(tc.tile_pool(name=R + "scr", bufs=2))
        escr = ctx.enter_context(tc.tile_pool(name=R + "escr", bufs=4))
        setup_ctx = ExitStack()
        psum_tp = setup_ctx.enter_context(
            tc.tile_pool(name=R + "psum_tp", bufs=2, space="PSUM")
        )
        psum_sm = setup_ctx.enter_context(
            tc.tile_pool(name=R + "psum_sm", bufs=2, space="PSUM")
        )
        dram = ctx.enter_context(tc.tile_pool(name=R + "dram", bufs=1, space="DRAM"))

        # ---- constants -------------------------------------------------
        lexcl_sb = const.tile([128, 128], F32, tag="lexcl")
        ident_sb = const.tile([128, 128], F32, tag="ident")
        rep16f_sb = const.tile([1, 2048], F32, tag="rep16f")
        rankbase_sb = const.tile([128, SLOTS * 8], F32, tag="rankbase")
        poff_sb = const.tile([128, 1], F32, tag="poff")
        wrow_sb = const.tile([128, NBLK], F32, tag="wrow")
        wfin_sb = const.tile([1, NBLK], F32, tag="wfin")
        ones_p = const.tile([128, 1], F32, tag="ones_p")    # column of ones
        ones_r = const.tile([1, 128], F32, tag="ones_r")    # row of ones
        for dst, src in [
            (lexcl_sb, lexcl),
            (ident_sb, ident),
            (rep16f_sb, rep16flat),
            (rankbase_sb, rankbase),
            (poff_sb, poff),
            (wrow_sb, wrow),
            (wfin_sb, wfin),
        ]:
            nc.sync.dma_start(dst[:], src[:])
        nc.vector.memset(ones_p[:], 1.0)
        nc.vector.memset(ones_r[:], 1.0)

        # ---- stage A: prob + thresholds --------------------------------
        prob_sb = main.tile([128, SLOTS, 128], F32, tag="prob")
        # prob_sb[p, c, f] = probT[c, p*128 + f]
        nc.sync.dma_start(
            prob_sb[:], probT.ap().rearrange("c (p f) -> p c f", p=128)
        )
        if "mini" in variant:
            inner = 4 if "4" in variant else 1
            for it in range(inner):
                mini_ps = psum_sm.tile([128, 128], F32, tag="sm128")
                nc.tensor.matmul(mini_ps[:], lexcl_sb[:], prob_sb[:, 0, :])
                mini = main.tile([1, 1], F32, tag=f"mini{it}")
                nc.vector.tensor_reduce(
                    mini[:], mini_ps[0:1, :], axis=mybir.AxisListType.X,
                    op=ALU.add,
                )
                nc.sync.dma_start(out[:], mini[:])
            setup_ctx.close()
            return
        taus = main.tile([1, 2 * SLOTS], F32, tag="taus")
        for c in range(SLOTS):
            nc.gpsimd.kth_largest(
                taus[0:1, 2 * c : 2 * c + 2],
                prob_sb[:, c, :],
                n_per_lane=128,
                k=K + 2,
                quantile=QUANTILE,
            )
        # broadcast tau (second output = 129th largest) to 128 partitions
        taub_ps = psum_sm.tile([128, SLOTS], F32, tag="sm")
        nc.tensor.matmul(taub_ps[:], ones_r[:], taus[0:1, 1 : 2 * SLOTS : 2])
        taub = main.tile([128, SLOTS], F32, tag="taub_sb")
        nc.vector.tensor_copy(taub[:], taub_ps[:])

        # ---- stage B: index extraction via max8 + local_scatter --------
        vals3 = main.tile([128, SLOTS, 8], F32, tag="vals3")
        idxs3 = main.tile([128, SLOTS, 8], U16, tag="idxs3")
        for c in range(SLOTS):
            nc.vector.max(vals3[:, c, :], prob_sb[:, c, :])
            nc.vector.max_index(idxs3[:, c, :], vals3[:, c, :], prob_sb[:, c, :])
        valid3 = scr.tile([128, SLOTS, 8], F32, tag="valid3")
        nc.vector.tensor_tensor(
            valid3[:],
            vals3[:],
            taub[:].rearrange("p (c o) -> p c o", o=1).to_broadcast([128, SLOTS, 8]),
            op=ALU.is_gt,
        )
        rowcnt = scr.tile([128, SLOTS], F32, tag="rowcnt")
        nc.vector.tensor_reduce(
            rowcnt[:], valid3[:], axis=mybir.AxisListType.X, op=ALU.add
        )
        rowoff_ps = psum_sm.tile([128, SLOTS], F32, tag="sm")
        nc.tensor.matmul(rowoff_ps[:], lexcl_sb[:], rowcnt[:])
        ranks = scr.tile([128, SLOTS, 8], F32, tag="ranks")
        nc.vector.tensor_tensor(
            ranks[:],
            rowoff_ps[:]
            .rearrange("p (c o) -> p c o", o=1)
            .to_broadcast([128, SLOTS, 8]),
            rankbase_sb[:].rearrange("p (c t) -> p c t", t=8),
            op=ALU.add,
        )
        nc.vector.tensor_tensor(ranks[:], ranks[:], valid3[:], op=ALU.mult)
        nc.vector.tensor_scalar_add(ranks[:], ranks[:], -1.0)
        rank16 = main.tile([128, SLOTS * 8], I16, tag="rank16")
        nc.vector.tensor_copy(rank16[:], ranks[:].rearrange("p c t -> p (c t)"))
        # global index = 128 * partition + local index
        gidxf = scr.tile([128, SLOTS * 8], F32, tag="gidxf")
        nc.vector.tensor_copy(gidxf[:], idxs3[:].rearrange("p c t -> p (c t)"))
        nc.vector.tensor_tensor(
            gidxf[:], gidxf[:], poff_sb[:].to_broadcast([128, SLOTS * 8]),
            op=ALU.add,
        )
        cand16 = main.tile([128, SLOTS * 8], I16, tag="cand16")
        nc.vector.tensor_copy(cand16[:], gidxf[:])
        evr16 = main.tile([128, SLOTS * 128], I16, tag="evr16")
        nc.gpsimd.local_scatter(
            evr16[:], cand16[:], rank16[:],
            channels=128, num_elems=SLOTS * 128, num_idxs=SLOTS * 8,
        )
        evrf = scr.tile([128, SLOTS * 128], F32, tag="evrf")
        nc.vector.tensor_copy(evrf[:], evr16[:])
        # compact to a single [1, 896] row (each column has one nonzero)
        allidx_ps = psum_sm.tile([1, SLOTS * 128], F32, tag="smwide")
        nc.tensor.matmul(allidx_ps[:, 0:512], ones_p[:], evrf[:, 0:512])
        nc.tensor.matmul(
            allidx_ps[:, 512 : SLOTS * 128], ones_p[:], evrf[:, 512 : SLOTS * 128]
        )
        allidx = main.tile([1, SLOTS * 128], F32, tag="allidx")
        nc.scalar.copy(allidx[:], allidx_ps[:])
        # redistribute into the 16-partition wrap dma_gather expects
        widx_ps = psum_sm.tile([128, 56], F32, tag="sm")
        av = allidx[:].rearrange("p (s m) -> p m s", m=16)
        for m in range(16):
            nc.tensor.matmul(
                widx_ps[:],
                rep16f_sb[0:1, 128 * m : 128 * (m + 1)],
                av[0:1, m, :],
                start=(m == 0),
                stop=(m == 15),
            )
        idxs_i16 = main.tile([128, 56], I16, tag="idxs")
        nc.vector.tensor_copy(idxs_i16[:], widx_ps[:])

        # ---- stage D: gather selected rows -----------------------------
        gi = main.tile([128, SLOTS, 128], F32, tag="gi")
        gj = main.tile([128, SLOTS, 128], F32, tag="gj")
        for g_sb, z in ((gi, z_i), (gj, z_j)):
            nc.gpsimd.dma_gather(
                g_sb[:],
                z.ap(),
                idxs_i16[:],
                num_idxs=SLOTS * 128,
                num_idxs_reg=SLOTS * 128,
                elem_size=D,
            )

        # ---- stage E: normalize + transpose into bf16 table ------------
        # 1/|z| = exp(-0.5 * ln(|z|^2)) keeps every activation in the
        # natural_log_exp_and_others table (no Sqrt -> no table reloads).
        sqs = main.tile([128, NBLK], F32, tag="sqs")
        for b in range(NBLK):
            s, h = b // 2, b % 2
            src = (gi if h == 0 else gj)[:, s, :]
            trash = scr.tile([128, 128], F32, tag="trash")
            nc.scalar.activation(
                trash[:], src, AF.Square, accum_out=sqs[:, b : b + 1]
            )
        lnv = scr.tile([128, NBLK], F32, tag="lnv")
        nc.scalar.activation(lnv[:], sqs[:], AF.Ln)
        rnw = main.tile([128, NBLK], F32, tag="rnw")
        nc.scalar.activation(rnw[:], lnv[:], AF.Exp, scale=-0.5)
        nc.vector.tensor_tensor(rnw[:], rnw[:], wrow_sb[:], op=ALU.mult)

        agin = [
            dram.tile([128, CH_SZ[k]], BF16, name=f"agin{k}") for k in range(4)
        ]
        agout = [
            dram.tile(
                [N_CORES * 128, CH_SZ[k]], BF16, addr_space="Shared",
                name=f"agout{k}",
            )
            for k in range(4)
        ]
        flatT = main.tile([128, TBL], BF16, tag="flatT")
        ag_fired = 0
        for b in range(NBLK):
            s, h = b // 2, b % 2
            src = (gi if h == 0 else gj)[:, s, :]
            diag = scr.tile([128, 128], F32, tag="diag")
            nc.vector.tensor_tensor(
                diag[:], ident_sb[:], rnw[:, b : b + 1].to_broadcast([128, 128]),
                op=ALU.mult,
            )
            tp_ps = psum_tp.tile([128, 128], F32, tag="tp")
            nc.tensor.matmul(tp_ps[:], src, diag[:])
            nc.scalar.copy(
                flatT[:, 256 * s + 128 * h : 256 * s + 128 * h + 128], tp_ps[:]
            )
            # fire AllGather chunks as soon as their columns are built
            while ag_fired < 4 and b == CH_DONE_B[ag_fired]:
                k = ag_fired
                if "noag" not in variant:
                    nc.sync.dma_start(
                        agin[k][:], flatT[:, CH_OFF[k] : CH_OFF[k] + CH_SZ[k]]
                    )
                    nc.gpsimd.collective_compute(
                        "AllGather",
                        ALU.bypass,
                        replica_groups=[list(range(N_CORES))],
                        ins=[agin[k].opt()],
                        outs=[agout[k].opt()],
                    )
                ag_fired += 1

        # ---- stage F: rotated reload of peer chunks --------------------
        if "noag" not in variant:
            pid = nc.partition_id()
            for k in range(4):
                for j in range(1, N_CORES):
                    rj = (pid + j) & 7
                    dst = GOFF[k] + (j - 1) * CH_SZ[k]
                    nc.sync.dma_start(
                        flatT[:, dst : dst + CH_SZ[k]],
                        agout[k][bass.ds(rj * 128, 128), :],
                    )
        else:
            # fill peer columns locally (wrong numerics, same sweep timing)
            for k in range(4):
                for j in range(1, N_CORES):
                    dst = GOFF[k] + (j - 1) * CH_SZ[k]
                    nc.sync.dma_start(
                        flatT[:, dst : dst + CH_SZ[k]],
                        flatT[:, CH_OFF[k] : CH_OFF[k] + CH_SZ[k]],
                    )

        # ---- stage G: sim sweep (supertile-major) ----------------------
        setup_ctx.close()
        psum_sim = ctx.enter_context(
            tc.tile_pool(name=R + "psum_sim", bufs=2, space="PSUM")
        )
        if "nosweep" in variant:
            nos = main.tile([1, 1], F32, tag="nos")
            nc.vector.tensor_reduce(
                nos[:], flatT[0:1, :], axis=mybir.AxisListType.X, op=ALU.add
            )
            nc.sync.dma_start(out[:], nos[:])
            return
        partials = main.tile([128, NBLK, NST], F32, tag="partials")
        own_t = main.tile([128, NBLK], F32, tag="own_t")
        pos_t = main.tile([128, NBLK], F32, tag="pos_t")
        for st in range(NST):
            for b in range(NBLK):
                s, h = b // 2, b % 2
                lhsT = flatT[:, 256 * s + 128 * h : 256 * s + 128 * h + 128]
                sim_ps = psum_sim.tile([128, 2048], F32, tag="sim")
                for q0, qs in ((0, 512), (512, 512), (1024, 512), (1536, 256)):
                    nc.tensor.matmul(
                        sim_ps[:, q0 : q0 + qs],
                        lhsT,
                        flatT[:, ST * st + q0 : ST * st + q0 + qs],
                    )
                e_sb = escr.tile([128, ST], BF16, tag="e")
                nc.scalar.activation(
                    e_sb[:],
                    sim_ps[:, 0:ST],
                    AF.Exp,
                    scale=1.0 / TEMP,
                    accum_out=partials[:, b, st : st + 1],
                )
                if st == 0:
                    off = 256 * s
                    nc.vector.tensor_reduce(
                        pos_t[:, b : b + 1],
                        e_sb[:, off : off + 128],
                        axis=mybir.AxisListType.X,
                        op=ALU.add,
                    )
                    nc.vector.tensor_reduce(
                        own_t[:, b : b + 1],
                        e_sb[:, off : off + 256],
                        axis=mybir.AxisListType.X,
                        op=ALU.add,
                    )

        # ---- stage H: reduce to one scalar -----------------------------
        totals = main.tile([128, NBLK], F32, tag="totals")
        nc.vector.tensor_reduce(
            totals[:], partials[:], axis=mybir.AxisListType.X, op=ALU.add
        )
        neg = scr.tile([128, NBLK], F32, tag="neg")
        nc.vector.scalar_tensor_tensor(
            neg[:], totals[:], float(-N_DUMMY_COLS), own_t[:],
            op0=ALU.add, op1=ALU.subtract,
        )
        lnn = scr.tile([128, NBLK], F32, tag="lnn")
        lnp = scr.tile([128, NBLK], F32, tag="lnp")
        nc.scalar.activation(lnn[:], neg[:], AF.Ln)
        nc.scalar.activation(lnp[:], pos_t[:], AF.Ln)
        lrows = main.tile([128, NBLK], F32, tag="lrows")
        nc.vector.tensor_sub(lrows[:], lnn[:], lnp[:])
        fin_ps = psum_sim.tile([1, NBLK], F32, tag="sim")
        nc.tensor.matmul(fin_ps[:], ones_p[:], lrows[:])
        fin_sb = main.tile([1, NBLK], F32, tag="fin_sb")
        nc.vector.tensor_tensor(fin_sb[:], fin_ps[:], wfin_sb[:], op=ALU.mult)
        out_sb = main.tile([1, 1], F32, tag="out_sb")
        nc.vector.tensor_reduce(
            out_sb[:], fin_sb[:], axis=mybir.AxisListType.X, op=ALU.add
        )
        nc.vector.tensor_scalar_mul(out_sb[:], out_sb[:], 1.0 / (2 * K * C))
        nc.sync.dma_start(out[:], out_sb[:])


def _per_core_inputs(prob, z_i, z_j):
    consts = _host_constants()
    maps = []
    for k in range(N_CORES):
        ncl = CCNT[k]
        cols = list(range(CBASE[k], CBASE[k] + ncl))
        cols = cols + [CBASE[k]] * (SLOTS - ncl)  # dummy slots reuse first col
        pT = np.ascontiguousarray(prob[:, cols].T)  # [SLOTS, B]
        w = np.array([1.0] * ncl + [0.0] * (SLOTS - ncl), dtype=np.float32)
        wrow = np.broadcast_to(
            np.repeat(w, 2)[None, :], (128, NBLK)
        ).copy()  # [128, 14]
        wfin = np.repeat(w, 2)[None, :].astype(np.float32).copy()  # [1, 14]
        m = {
            "probT": pT,
            "z_i": z_i,
            "z_j": z_j,
            "wrow": wrow,
            "wfin": wfin,
        }
        m.update(consts)
        maps.append(m)
    return maps


def kernel(prob, z_i, z_j):
    if "nc" not in _CACHE:
        _CACHE["nc"] = _build_program()
    nc = _CACHE["nc"]
    in_maps = _per_core_inputs(
        np.asarray(prob, dtype=np.float32),
        np.ascontiguousarray(z_i, dtype=np.float32),
        np.ascontiguousarray(z_j, dtype=np.float32),
    )
    res = run_bass_kernel_spmd(nc, in_maps, list(range(N_CORES)))
    total = np.float32(0.0)
    for r in res.results:
        total += r["partial0"][0, 0]
    return np.asarray(total, dtype=np.float32)


# revision 14
# speedup vs baseline: 32.8893x; 30.0198x over previous
"""Cluster-based contrastive loss on 8 Trainium2 NeuronCores.

Strategy: shard the C=50 cluster axis across 8 cores (7 slots/core, padded
to 56 global slots; 6 dummy slots carry weight 0).  Each core:
  - computes exact top-128 thresholds for its clusters (gpsimd kth_largest
    returns the 129th-largest value exactly),
  - finds per-partition top-8 values+indices (DVE max/max_index), ranks the
    survivors with a prefix-sum matmul and compacts them with a gpsimd
    local_scatter (one-hot matmul eliminated),
  - gathers the selected z_i / z_j rows from HBM (dma_gather),
  - normalizes (exp(-0.5*ln(|z|^2)) so the whole kernel uses one ACT
    function table) + transposes into a bf16 [128, D->partition] column
    table via a PE matmul against diag(w / ||z||),
  - AllGathers the table in 4 column-chunks (fired as soon as each chunk
    is built) so the collective overlaps the sim sweep,
  - sweeps flatT.T @ flatT supertile-major (1792 cols per supertile) with
    ACT exp (accum_out gives row sums free), subtracts the own-cluster
    block, and reduces log(neg) - log(pos) to one scalar.
The host sums the 8 per-core partial scalars.
"""

import sys

sys.path.insert(0, "/opt/trn_rl_repo")

import numpy as np

import concourse.bacc as bacc
import concourse.bass as bass
import concourse.mybir as mybir
from concourse import tile
from concourse.bass_utils import run_bass_kernel_spmd

F32 = mybir.dt.float32
BF16 = mybir.dt.bfloat16
I16 = mybir.dt.int16
U16 = mybir.dt.uint16
AF = mybir.ActivationFunctionType
ALU = mybir.AluOpType

B = 16384
D = 128
C = 50
K = 128
TEMP = 0.5
N_CORES = 8
SLOTS = 7                      # cluster slots per core
GSLOTS = N_CORES * SLOTS       # 56 global slots
TBL = GSLOTS * 2 * K           # 14336 columns in the padded table
LOCAL = SLOTS * 2 * K          # 1792 columns contributed per core
NBLK = SLOTS * 2               # 14 row blocks per core
N_DUMMY_COLS = (GSLOTS - C) * 2 * K   # 1536 zero columns in the table
# core k owns clusters [CBASE[k], CBASE[k] + CCNT[k])
CCNT = [7, 7, 6, 6, 6, 6, 6, 6]
CBASE = [0, 7, 14, 20, 26, 32, 38, 44]
QUANTILE = 1.0 - 127.5 / (B - 1)
# AllGather column chunks of the local table (fired as stage E completes
# each range); group k of the sweep table holds all 7 peers' chunk k.
CH_OFF = [0, 512, 1024, 1536]
CH_SZ = [512, 512, 512, 256]
CH_DONE_B = [3, 7, 11, 13]     # last stage-E block (2s+h) filling each chunk
GOFF = [1792, 5376, 8960, 12544]
ST = 1792                      # sweep supertile width; TBL = 8 * ST
NST = TBL // ST

_CACHE = {}


def _host_constants():
    lexcl = (np.arange(128)[:, None] < np.arange(128)[None, :]).astype(np.float32)
    ident = np.eye(128, dtype=np.float32)
    # rep16flat[0, 128*m + j] = (j % 16 == m): outer-product rows used to
    # redistribute the compacted [1, 896] index row into 16-partition wrap.
    rep16flat = np.zeros((1, 16 * 128), dtype=np.float32)
    for m in range(16):
        rep16flat[0, 128 * m + np.arange(m, 128, 16)] = 1.0
    # rankbase[p, 8c+t] = 128*c + t + 1  (cluster-global scatter slot, +1 so
    # invalid lanes become -1 after (x * valid) - 1)
    rb = (128 * np.arange(SLOTS)[:, None] + np.arange(8)[None, :] + 1).astype(
        np.float32
    )
    rankbase = np.broadcast_to(rb.reshape(1, SLOTS * 8), (128, SLOTS * 8)).copy()
    poff = (128.0 * np.arange(128, dtype=np.float32))[:, None].copy()
    return {
        "lexcl": lexcl,
        "ident": ident,
        "rep16flat": rep16flat,
        "rankbase": rankbase,
        "poff": poff,
    }


def _build_program(repeats=1, variant="full"):
    nc = bacc.Bacc(
        "TRN2", target_bir_lowering=False, debug=False, num_devices=N_CORES
    )

    probT = nc.dram_tensor("probT", [SLOTS, B], F32, kind="ExternalInput")
    z_i = nc.dram_tensor("z_i", [B, D], F32, kind="ExternalInput")
    z_j = nc.dram_tensor("z_j", [B, D], F32, kind="ExternalInput")
    wrow = nc.dram_tensor("wrow", [128, NBLK], F32, kind="ExternalInput")
    wfin = nc.dram_tensor("wfin", [1, NBLK], F32, kind="ExternalInput")
    lexcl = nc.dram_tensor("lexcl", [128, 128], F32, kind="ExternalInput")
    ident = nc.dram_tensor("ident", [128, 128], F32, kind="ExternalInput")
    rep16flat = nc.dram_tensor("rep16flat", [1, 2048], F32, kind="ExternalInput")
    rankbase = nc.dram_tensor("rankbase", [128, SLOTS * 8], F32, kind="ExternalInput")
    poff = nc.dram_tensor("poff", [128, 1], F32, kind="ExternalInput")
    outs = [
        nc.dram_tensor(f"partial{r}", [1, 1], F32, kind="ExternalOutput")
        for r in range(repeats)
    ]

    with tile.TileContext(nc) as tc:
        for r in range(repeats):
            _emit(
                nc, tc, probT, z_i, z_j, wrow, wfin, lexcl, ident, rep16flat,
                rankbase, poff, outs[r], rep=r, variant=variant,
            )

    nc.compile()
    return nc


def _emit(nc, tc, probT, z_i, z_j, wrow, wfin, lexcl, ident, rep16flat,
          rankbase, poff, out, rep=0, variant="full"):
    from contextlib import ExitStack

    R = f"r{rep}_"
    ctx = ExitStack()
    with ctx:
        const = ctx.enter_context(tc.tile_pool(name=R + "const", bufs=1))
        main = ctx.enter_context(tc.tile_pool(name=R + "main", bufs=1))
        scr = ctx.enter_context(tc.tile_pool(name=R + "scr", bufs=2))
        escr = ctx.enter_context(tc.tile_pool(name=R + "escr", bufs=4))
        setup_ctx = ExitStack()
        psum_tp = setup_ctx.enter_context(
            tc.tile_pool(name=R + "psum_tp", bufs=2, space="PSUM")
        )
        psum_sm = setup_ctx.enter_context(
            tc.tile_pool(name=R + "psum_sm", bufs=2, space="PSUM")
        )
        dram = ctx.enter_context(tc.tile_pool(name=R + "dram", bufs=1, space="DRAM"))

        # ---- constants -------------------------------------------------
        lexcl_sb = const.tile([128, 128], F32, tag="lexcl")
        ident_sb = const.tile([128, 128], F32, tag="ident")
        rep16f_sb = const.tile([1, 2048], F32, tag="rep16f")
        rankbase_sb = const.tile([128, SLOTS * 8], F32, tag="rankbase")
        poff_sb = const.tile([128, 1], F32, tag="poff")
        wrow_sb = const.tile([128, NBLK], F32, tag="wrow")
        wfin_sb = const.tile([1, NBLK], F32, tag="wfin")
        ones_p = const.tile([128, 1], F32, tag="ones_p")    # column of ones
        ones_r = const.tile([1, 128], F32, tag="ones_r")    # row of ones
        const_srcs = [
            (lexcl_sb, lexcl),
            (ident_sb, ident),
            (rep16f_sb, rep16flat),
            (rankbase_sb, rankbase),
            (poff_sb, poff),
            (wrow_sb, wrow),
            (wfin_sb, wfin),
        ]
        if "noconst" in variant:
            for dst, _ in const_srcs:
                nc.vector.memset(dst[:], 0.5)
        else:
            for dst, src in const_srcs:
                nc.sync.dma_start(dst[:], src[:])
        nc.vector.memset(ones_p[:], 1.0)
        nc.vector.memset(ones_r[:], 1.0)

        # ---- stage A: prob + thresholds --------------------------------
        prob_sb = main.tile([128, SLOTS, 128], F32, tag="prob")
        # prob_sb[p, c, f] = probT[c, p*128 + f]
        if "noprob" in variant:
            nc.vector.memset(prob_sb[:], 0.25)
        else:
            nc.sync.dma_start(
                prob_sb[:], probT.ap().rearrange("c (p f) -> p c f", p=128)
            )
        if "mini" in variant:
            inner = 4 if "4" in variant else 1
            for it in range(inner):
                mini_ps = psum_sm.tile([128, 128], F32, tag="sm128")
                nc.tensor.matmul(mini_ps[:], lexcl_sb[:], prob_sb[:, 0, :])
                mini = main.tile([1, 1], F32, tag=f"mini{it}")
                nc.vector.tensor_reduce(
                    mini[:], mini_ps[0:1, :], axis=mybir.AxisListType.X,
                    op=ALU.add,
                )
                nc.sync.dma_start(out[:], mini[:])
            setup_ctx.close()
            return
        taus = main.tile([1, 2 * SLOTS], F32, tag="taus")
        for c in range(SLOTS):
            nc.gpsimd.kth_largest(
                taus[0:1, 2 * c : 2 * c + 2],
                prob_sb[:, c, :],
                n_per_lane=128,
                k=K + 2,
                quantile=QUANTILE,
            )
        # broadcast tau (second output = 129th largest) to 128 partitions
        taub_ps = psum_sm.tile([128, SLOTS], F32, tag="sm")
        nc.tensor.matmul(taub_ps[:], ones_r[:], taus[0:1, 1 : 2 * SLOTS : 2])
        taub = main.tile([128, SLOTS], F32, tag="taub_sb")
        nc.vector.tensor_copy(taub[:], taub_ps[:])

        # ---- stage B: index extraction via max8 + local_scatter --------
        vals3 = main.tile([128, SLOTS, 8], F32, tag="vals3")
        idxs3 = main.tile([128, SLOTS, 8], U16, tag="idxs3")
        for c in range(SLOTS):
            nc.vector.max(vals3[:, c, :], prob_sb[:, c, :])
            nc.vector.max_index(idxs3[:, c, :], vals3[:, c, :], prob_sb[:, c, :])
        valid3 = scr.tile([128, SLOTS, 8], F32, tag="valid3")
        nc.vector.tensor_tensor(
            valid3[:],
            vals3[:],
            taub[:].rearrange("p (c o) -> p c o", o=1).to_broadcast([128, SLOTS, 8]),
            op=ALU.is_gt,
        )
        rowcnt = scr.tile([128, SLOTS], F32, tag="rowcnt")
        nc.vector.tensor_reduce(
            rowcnt[:], valid3[:], axis=mybir.AxisListType.X, op=ALU.add
        )
        rowoff_ps = psum_sm.tile([128, SLOTS], F32, tag="sm")
        nc.tensor.matmul(rowoff_ps[:], lexcl_sb[:], rowcnt[:])
        ranks = scr.tile([128, SLOTS, 8], F32, tag="ranks")
        nc.vector.tensor_tensor(
            ranks[:],
            rowoff_ps[:]
            .rearrange("p (c o) -> p c o", o=1)
            .to_broadcast([128, SLOTS, 8]),
            rankbase_sb[:].rearrange("p (c t) -> p c t", t=8),
            op=ALU.add,
        )
        nc.vector.tensor_tensor(ranks[:], ranks[:], valid3[:], op=ALU.mult)
        nc.vector.tensor_scalar_add(ranks[:], ranks[:], -1.0)
        rank16 = main.tile([128, SLOTS * 8], I16, tag="rank16")
        nc.vector.tensor_copy(rank16[:], ranks[:].rearrange("p c t -> p (c t)"))
        # global index = 128 * partition + local index
        gidxf = scr.tile([128, SLOTS * 8], F32, tag="gidxf")
        nc.vector.tensor_copy(gidxf[:], idxs3[:].rearrange("p c t -> p (c t)"))
        nc.vector.tensor_tensor(
            gidxf[:], gidxf[:], poff_sb[:].to_broadcast([128, SLOTS * 8]),
            op=ALU.add,
        )
        cand16 = main.tile([128, SLOTS * 8], I16, tag="cand16")
        nc.vector.tensor_copy(cand16[:], gidxf[:])
        evr16 = main.tile([128, SLOTS * 128], I16, tag="evr16")
        nc.gpsimd.local_scatter(
            evr16[:], cand16[:], rank16[:],
            channels=128, num_elems=SLOTS * 128, num_idxs=SLOTS * 8,
        )
        evrf = scr.tile([128, SLOTS * 128], F32, tag="evrf")
        nc.vector.tensor_copy(evrf[:], evr16[:])
        # compact to a single [1, 896] row (each column has one nonzero)
        allidx_ps = psum_sm.tile([1, SLOTS * 128], F32, tag="smwide")
        nc.tensor.matmul(allidx_ps[:, 0:512], ones_p[:], evrf[:, 0:512])
        nc.tensor.matmul(
            allidx_ps[:, 512 : SLOTS * 128], ones_p[:], evrf[:, 512 : SLOTS * 128]
        )
        allidx = main.tile([1, SLOTS * 128], F32, tag="allidx")
        nc.scalar.copy(allidx[:], allidx_ps[:])
        # redistribute into the 16-partition wrap dma_gather expects
        widx_ps = psum_sm.tile([128, 56], F32, tag="sm")
        av = allidx[:].rearrange("p (s m) -> p m s", m=16)
        for m in range(16):
            nc.tensor.matmul(
                widx_ps[:],
                rep16f_sb[0:1, 128 * m : 128 * (m + 1)],
                av[0:1, m, :],
                start=(m == 0),
                stop=(m == 15),
            )
        idxs_i16 = main.tile([128, 56], I16, tag="idxs")
        nc.vector.tensor_copy(idxs_i16[:], widx_ps[:])

        # ---- stage D: gather selected rows -----------------------------
        gi = main.tile([128, SLOTS, 128], F32, tag="gi")
        gj = main.tile([128, SLOTS, 128], F32, tag="gj")
        for g_sb, z in ((gi, z_i), (gj, z_j)):
            nc.gpsimd.dma_gather(
                g_sb[:],
                z.ap(),
                idxs_i16[:],
                num_idxs=SLOTS * 128,
                num_idxs_reg=SLOTS * 128,
                elem_size=D,
            )

        # ---- stage E: normalize + transpose into bf16 table ------------
        # 1/|z| = exp(-0.5 * ln(|z|^2)) keeps every activation in the
        # natural_log_exp_and_others table (no Sqrt -> no table reloads).
        sqs = main.tile([128, NBLK], F32, tag="sqs")
        for b in range(NBLK):
            s, h = b // 2, b % 2
            src = (gi if h == 0 else gj)[:, s, :]
            trash = scr.tile([128, 128], F32, tag="trash")
            nc.scalar.activation(
                trash[:], src, AF.Square, accum_out=sqs[:, b : b + 1]
            )
        lnv = scr.tile([128, NBLK], F32, tag="lnv")
        nc.scalar.activation(lnv[:], sqs[:], AF.Ln)
        rnw = main.tile([128, NBLK], F32, tag="rnw")
        nc.scalar.activation(rnw[:], lnv[:], AF.Exp, scale=-0.5)
        nc.vector.tensor_tensor(rnw[:], rnw[:], wrow_sb[:], op=ALU.mult)

        agin = [
            dram.tile([128, CH_SZ[k]], BF16, name=f"agin{k}") for k in range(4)
        ]
        agout = [
            dram.tile(
                [N_CORES * 128, CH_SZ[k]], BF16, addr_space="Shared",
                name=f"agout{k}",
            )
            for k in range(4)
        ]
        flatT = main.tile([128, TBL], BF16, tag="flatT")
        ag_fired = 0
        for b in range(NBLK):
            s, h = b // 2, b % 2
            src = (gi if h == 0 else gj)[:, s, :]
            diag = scr.tile([128, 128], F32, tag="diag")
            nc.vector.tensor_tensor(
                diag[:], ident_sb[:], rnw[:, b : b + 1].to_broadcast([128, 128]),
                op=ALU.mult,
            )
            tp_ps = psum_tp.tile([128, 128], F32, tag="tp")
            nc.tensor.matmul(tp_ps[:], src, diag[:])
            nc.scalar.copy(
                flatT[:, 256 * s + 128 * h : 256 * s + 128 * h + 128], tp_ps[:]
            )
            # fire AllGather chunks as soon as their columns are built
            while ag_fired < 4 and b == CH_DONE_B[ag_fired]:
                k = ag_fired
                if "noag" not in variant:
                    nc.sync.dma_start(
                        agin[k][:], flatT[:, CH_OFF[k] : CH_OFF[k] + CH_SZ[k]]
                    )
                    nc.gpsimd.collective_compute(
                        "AllGather",
                        ALU.bypass,
                        replica_groups=[list(range(N_CORES))],
                        ins=[agin[k].opt()],
                        outs=[agout[k].opt()],
                    )
                ag_fired += 1

        # ---- stage F: rotated reload of peer chunks --------------------
        if "noag" not in variant:
            pid = nc.partition_id()
            for k in range(4):
                for j in range(1, N_CORES):
                    rj = (pid + j) & 7
                    dst = GOFF[k] + (j - 1) * CH_SZ[k]
                    nc.sync.dma_start(
                        flatT[:, dst : dst + CH_SZ[k]],
                        agout[k][bass.ds(rj * 128, 128), :],
                    )
        else:
            # fill peer columns locally (wrong numerics, same sweep timing)
            for k in range(4):
                for j in range(1, N_CORES):
                    dst = GOFF[k] + (j - 1) * CH_SZ[k]
                    nc.sync.dma_start(
                        flatT[:, dst : dst + CH_SZ[k]],
                        flatT[:, CH_OFF[k] : CH_OFF[k] + CH_SZ[k]],
                    )

        # ---- stage G: sim sweep (supertile-major) ----------------------
        setup_ctx.close()
        psum_sim = ctx.enter_context(
            tc.tile_pool(name=R + "psum_sim", bufs=2, space="PSUM")
        )
        if "nosweep" in variant:
            nos = main.tile([1, 1], F32, tag="nos")
            nc.vector.tensor_reduce(
                nos[:], flatT[0:1, :], axis=mybir.AxisListType.X, op=ALU.add
            )
            nc.sync.dma_start(out[:], nos[:])
            return
        partials = main.tile([128, NBLK, NST], F32, tag="partials")
        own_t = main.tile([128, NBLK], F32, tag="own_t")
        pos_t = main.tile([128, NBLK], F32, tag="pos_t")
        for st in range(NST):
            for b in range(NBLK):
                s, h = b // 2, b % 2
                lhsT = flatT[:, 256 * s + 128 * h : 256 * s + 128 * h + 128]
                sim_ps = psum_sim.tile([128, 2048], F32, tag="sim")
                for q0, qs in ((0, 512), (512, 512), (1024, 512), (1536, 256)):
                    nc.tensor.matmul(
                        sim_ps[:, q0 : q0 + qs],
                        lhsT,
                        flatT[:, ST * st + q0 : ST * st + q0 + qs],
                    )
                e_sb = escr.tile([128, ST], BF16, tag="e")
                nc.scalar.activation(
                    e_sb[:],
                    sim_ps[:, 0:ST],
                    AF.Exp,
                    scale=1.0 / TEMP,
                    accum_out=partials[:, b, st : st + 1],
                )
                if st == 0:
                    off = 256 * s
                    nc.vector.tensor_reduce(
                        pos_t[:, b : b + 1],
                        e_sb[:, off : off + 128],
                        axis=mybir.AxisListType.X,
                        op=ALU.add,
                    )
                    nc.vector.tensor_reduce(
                        own_t[:, b : b + 1],
                        e_sb[:, off : off + 256],
                        axis=mybir.AxisListType.X,
                        op=ALU.add,
                    )

        # ---- stage H: reduce to one scalar -----------------------------
        totals = main.tile([128, NBLK], F32, tag="totals")
        nc.vector.tensor_reduce(
            totals[:], partials[:], axis=mybir.AxisListType.X, op=ALU.add
        )
        neg = scr.tile([128, NBLK], F32, tag="neg")
        nc.vector.scalar_tensor_tensor(
            neg[:], totals[:], float(-N_DUMMY_COLS), own_t[:],
            op0=ALU.add, op1=ALU.subtract,
        )
        lnn = scr.tile([128, NBLK], F32, tag="lnn")
        lnp = scr.tile([128, NBLK], F32, tag="lnp")
        nc.scalar.activation(lnn[:], neg[:], AF.Ln)
        nc.scalar.activation(lnp[:], pos_t[:], AF.Ln)
        lrows = main.tile([128, NBLK], F32, tag="lrows")
        nc.vector.tensor_sub(lrows[:], lnn[:], lnp[:])
        fin_ps = psum_sim.tile([1, NBLK], F32, tag="sim")
        nc.tensor.matmul(fin_ps[:], ones_p[:], lrows[:])
        fin_sb = main.tile([1, NBLK], F32, tag="fin_sb")
        nc.vector.tensor_tensor(fin_sb[:], fin_ps[:], wfin_sb[:], op=ALU.mult)
        out_sb = main.tile([1, 1], F32, tag="out_sb")
        nc.vector.tensor_reduce(
            out_sb[:], fin_sb[:], axis=mybir.AxisListType.X, op=ALU.add
        )
        nc.vector.tensor_scalar_mul(out_sb[:], out_sb[:], 1.0 / (2 * K * C))
        nc.sync.dma_start(out[:], out_sb[:])


def _per_core_inputs(prob, z_i, z_j):
    consts = _host_constants()
    maps = []
    for k in range(N_CORES):
        ncl = CCNT[k]
        cols = list(range(CBASE[k], CBASE[k] + ncl))
        cols = cols + [CBASE[k]] * (SLOTS - ncl)  # dummy slots reuse first col
        pT = np.ascontiguousarray(prob[:, cols].T)  # [SLOTS, B]
        w = np.array([1.0] * ncl + [0.0] * (SLOTS - ncl), dtype=np.float32)
        wrow = np.broadcast_to(
            np.repeat(w, 2)[None, :], (128, NBLK)
        ).copy()  # [128, 14]
        wfin = np.repeat(w, 2)[None, :].astype(np.float32).copy()  # [1, 14]
        m = {
            "probT": pT,
            "z_i": z_i,
            "z_j": z_j,
            "wrow": wrow,
            "wfin": wfin,
        }
        m.update(consts)
        maps.append(m)
    return maps


def kernel(prob, z_i, z_j):
    if "nc" not in _CACHE:
        _CACHE["nc"] = _build_program()
    nc = _CACHE["nc"]
    in_maps = _per_core_inputs(
        np.asarray(prob, dtype=np.float32),
        np.ascontiguousarray(z_i, dtype=np.float32),
        np.ascontiguousarray(z_j, dtype=np.float32),
    )
    res = run_bass_kernel_spmd(nc, in_maps, list(range(N_CORES)))
    total = np.float32(0.0)
    for r in res.results:
        total += r["partial0"][0, 0]
    return np.asarray(total, dtype=np.float32)


# revision 27
# speedup vs baseline: 42.2192x; 1.2837x over previous
"""Cluster-based contrastive loss on 8 Trainium2 NeuronCores.

Strategy: shard the C=50 cluster axis across 8 cores (7 slots/core, padded
to 56 global slots; 6 dummy slots carry weight 0).  Each core:
  - computes exact top-128 thresholds for its clusters (gpsimd kth_largest),
  - finds per-partition top-8 values+indices (DVE max/max_index), ranks the
    survivors with a prefix-sum matmul and compacts them with a gpsimd
    local_scatter,
  - gathers all 1792 selected rows from a host-concatenated z=[2B,D] table
    with ONE dma_gather (z_j indices are z_i indices + B),
  - normalizes (exp(-0.5*ln(|z|^2)): the whole kernel stays on one ACT
    function table) + transposes into a bf16 [128, D->partition] column
    table via PE matmuls against diag(w / ||z||),
  - AllGathers the table in 2 column-chunks (fired as soon as built) so the
    collective overlaps the start of the sim sweep,
  - sweeps flatT.T @ flatT in 2048-col supertiles (one wide matmul + one
    ACT exp with accum_out per supertile), subtracts the own-cluster block,
    reduces log(neg) - log(pos) to one scalar.
The host sums the 8 per-core partial scalars.

Instruction count is deliberately minimized (the dominant HW cost here is
per-instruction dispatch, not engine throughput): packed constants, host
pre-transposed prob, wide matmuls, batched DVE ops.
"""

import sys

sys.path.insert(0, "/opt/trn_rl_repo")

import numpy as np

import concourse.bacc as bacc
import concourse.bass as bass
import concourse.mybir as mybir
from concourse import tile
from concourse.bass_utils import run_bass_kernel_spmd

F32 = mybir.dt.float32
BF16 = mybir.dt.bfloat16
I16 = mybir.dt.int16
U16 = mybir.dt.uint16
AF = mybir.ActivationFunctionType
ALU = mybir.AluOpType

B = 16384
D = 128
C = 50
K = 128
TEMP = 0.5
N_CORES = 8
SLOTS = 7                      # cluster slots per core
GSLOTS = N_CORES * SLOTS       # 56 global slots
TBL = GSLOTS * 2 * K           # 14336 columns in the padded table
LOCAL = SLOTS * 2 * K          # 1792 columns contributed per core
NBLK = SLOTS * 2               # 14 row blocks per core
N_DUMMY_COLS = (GSLOTS - C) * 2 * K   # 1536 zero columns in the table
# core k owns clusters [CBASE[k], CBASE[k] + CCNT[k])
CCNT = [7, 7, 6, 6, 6, 6, 6, 6]
CBASE = [0, 7, 14, 20, 26, 32, 38, 44]
QUANTILE = 1.0 - 127.5 / (B - 1)
# AllGather column chunks of the local table; group k of the sweep table
# holds all 7 peers' chunk k at GOFF[k].
CH_OFF = [0, 1024]
CH_SZ = [1024, 768]
CH_DONE_N = [8, 14]            # stage-E blocks (2s+h order) filling each chunk
GOFF = [1792, 8960]
ST = 2048                      # sweep supertile width; TBL = 7 * ST
NST = TBL // ST
MM_CHUNK = 512                 # matmul out chunk (ISA cap: one PSUM bank)
# packed big-constant layout (columns)
BC_LEXCL = 0
BC_IDENT = 128
BC_RANKB = 256
BC_POFF = 312
BC_WROW = 313
BC_COLS = 327

_CACHE = {}


def _host_constants():
    bigc = np.zeros((128, BC_COLS), dtype=np.float32)
    bigc[:, BC_LEXCL : BC_LEXCL + 128] = (
        np.arange(128)[:, None] < np.arange(128)[None, :]
    )
    bigc[:, BC_IDENT : BC_IDENT + 128] = np.eye(128)
    # rankbase[p, 8c+t] = 128*c + t + 1 (+1 so invalid -> -1 after *valid -1)
    rb = (128 * np.arange(SLOTS)[:, None] + np.arange(8)[None, :] + 1).astype(
        np.float32
    )
    bigc[:, BC_RANKB : BC_RANKB + 56] = rb.reshape(1, 56)
    bigc[:, BC_POFF] = 128.0 * np.arange(128)
    # rep16flat[0, 128*m + j] = (j % 16 == m)
    rep16flat = np.zeros((1, 16 * 128), dtype=np.float32)
    for m in range(16):
        rep16flat[0, 128 * m + np.arange(m, 128, 16)] = 1.0
    return bigc, rep16flat


def _build_program(repeats=1, variant="full"):
    nc = bacc.Bacc(
        "TRN2", target_bir_lowering=False, debug=False, num_devices=N_CORES
    )

    probw = nc.dram_tensor("probw", [128, SLOTS * 128], F32, kind="ExternalInput")
    z = nc.dram_tensor("z", [2 * B, D], F32, kind="ExternalInput")
    bigc = nc.dram_tensor("bigc", [128, BC_COLS], F32, kind="ExternalInput")
    rep16flat = nc.dram_tensor("rep16flat", [1, 2048], F32, kind="ExternalInput")
    wfin = nc.dram_tensor("wfin", [1, NBLK], F32, kind="ExternalInput")
    outs = [
        nc.dram_tensor(f"partial{r}", [1, 1], F32, kind="ExternalOutput")
        for r in range(repeats)
    ]

    with tile.TileContext(nc) as tc:
        for r in range(repeats):
            _emit(nc, tc, probw, z, bigc, rep16flat, wfin, outs[r], rep=r,
                  variant=variant)

    nc.compile()
    return nc


def _emit(nc, tc, probw, z, bigc, rep16flat, wfin, out, rep=0, variant="full"):
    from contextlib import ExitStack

    R = f"r{rep}_"
    ctx = ExitStack()
    with ctx:
        const = ctx.enter_context(tc.tile_pool(name=R + "const", bufs=1))
        main = ctx.enter_context(tc.tile_pool(name=R + "main", bufs=1))
        scr = ctx.enter_context(tc.tile_pool(name=R + "scr", bufs=2))
        escr = ctx.enter_context(tc.tile_pool(name=R + "escr", bufs=4))
        setup_ctx = ExitStack()
        psum_tp = setup_ctx.enter_context(
            tc.tile_pool(name=R + "psum_tp", bufs=2, space="PSUM")
        )
        psum_sm = setup_ctx.enter_context(
            tc.tile_pool(name=R + "psum_sm", bufs=2, space="PSUM")
        )
        dram = ctx.enter_context(tc.tile_pool(name=R + "dram", bufs=1, space="DRAM"))

        # ---- constants (3 DMAs) ----------------------------------------
        bigc_sb = const.tile([128, BC_COLS], F32, tag="bigc")
        rep16f_sb = const.tile([1, 2048], F32, tag="rep16f")
        wfin_sb = const.tile([1, NBLK], F32, tag="wfin")
        nc.sync.dma_start(bigc_sb[:], bigc.ap())
        nc.sync.dma_start(rep16f_sb[:], rep16flat.ap())
        nc.sync.dma_start(wfin_sb[:], wfin.ap())
        lexcl_sb = bigc_sb[:, BC_LEXCL : BC_LEXCL + 128]
        ident_sb = bigc_sb[:, BC_IDENT : BC_IDENT + 128]
        rankb_sb = bigc_sb[:, BC_RANKB : BC_RANKB + 56]
        poff_sb = bigc_sb[:, BC_POFF : BC_POFF + 1]
        wrow_sb = bigc_sb[:, BC_WROW : BC_WROW + NBLK]
        ones_p = const.tile([128, 1], F32, tag="ones_p")
        ones_r = const.tile([1, 128], F32, tag="ones_r")
        nc.vector.memset(ones_p[:], 1.0)
        nc.vector.memset(ones_r[:], 1.0)

        # ---- stage A: prob + thresholds --------------------------------
        prob_sb = main.tile([128, SLOTS, 128], F32, tag="prob")
        nc.sync.dma_start(prob_sb[:], probw.ap())
        if "mini" in variant:
            mini_ps = psum_sm.tile([128, 128], F32, tag="sm128")
            nc.tensor.matmul(mini_ps[:], lexcl_sb, prob_sb[:, 0, :])
            mini = main.tile([1, 1], F32, tag="mini")
            nc.vector.tensor_reduce(
                mini[:], mini_ps[0:1, :], axis=mybir.AxisListType.X, op=ALU.add
            )
            nc.sync.dma_start(out[:], mini[:])
            setup_ctx.close()
            return
        taus = main.tile([1, 2 * SLOTS], F32, tag="taus")
        for c in range(SLOTS):
            nc.gpsimd.kth_largest(
                taus[0:1, 2 * c : 2 * c + 2],
                prob_sb[:, c, :],
                n_per_lane=128,
                k=K + 2,
                quantile=QUANTILE,
            )
        taub_ps = psum_sm.tile([128, SLOTS], F32, tag="sm")
        nc.tensor.matmul(taub_ps[:], ones_r[:], taus[0:1, 1 : 2 * SLOTS : 2])

        def _stop(src_ap):
            stp = main.tile([1, 1], F32, tag="stop")
            nc.vector.tensor_reduce(
                stp[:], src_ap, axis=mybir.AxisListType.X, op=ALU.add
            )
            nc.sync.dma_start(out[:], stp[:])
            setup_ctx.close()

        if "stopA" in variant:
            _stop(taus[0:1, :])
            return

        # ---- stage B: index extraction via max8 + local_scatter --------
        vals3 = main.tile([128, SLOTS, 8], F32, tag="vals3")
        idxs3 = main.tile([128, SLOTS, 8], U16, tag="idxs3")
        for c in range(SLOTS):
            nc.vector.max(vals3[:, c, :], prob_sb[:, c, :])
            nc.vector.max_index(idxs3[:, c, :], vals3[:, c, :], prob_sb[:, c, :])
        valid3 = scr.tile([128, SLOTS, 8], F32, tag="valid3")
        nc.vector.tensor_tensor(
            valid3[:],
            vals3[:],
            taub_ps[:]
            .rearrange("p (c o) -> p c o", o=1)
            .to_broadcast([128, SLOTS, 8]),
            op=ALU.is_gt,
        )
        rowcnt = scr.tile([128, SLOTS], F32, tag="rowcnt")
        nc.vector.tensor_reduce(
            rowcnt[:], valid3[:], axis=mybir.AxisListType.X, op=ALU.add
        )
        rowoff_ps = psum_sm.tile([128, SLOTS], F32, tag="sm")
        nc.tensor.matmul(rowoff_ps[:], lexcl_sb, rowcnt[:])
        ranks = scr.tile([128, SLOTS, 8], F32, tag="ranks")
        nc.vector.tensor_tensor(
            ranks[:],
            rowoff_ps[:]
            .rearrange("p (c o) -> p c o", o=1)
            .to_broadcast([128, SLOTS, 8]),
            rankb_sb.rearrange("p (c t) -> p c t", t=8),
            op=ALU.add,
        )
        nc.vector.tensor_tensor(ranks[:], ranks[:], valid3[:], op=ALU.mult)
        nc.vector.tensor_scalar_add(ranks[:], ranks[:], -1.0)
        rank16 = main.tile([128, SLOTS * 8], I16, tag="rank16")
        nc.vector.tensor_copy(rank16[:], ranks[:].rearrange("p c t -> p (c t)"))
        gidxf = scr.tile([128, SLOTS * 8], F32, tag="gidxf")
        nc.vector.tensor_copy(gidxf[:], idxs3[:].rearrange("p c t -> p (c t)"))
        nc.vector.tensor_tensor(
            gidxf[:], gidxf[:], poff_sb.to_broadcast([128, SLOTS * 8]),
            op=ALU.add,
        )
        cand16 = main.tile([128, SLOTS * 8], I16, tag="cand16")
        nc.vector.tensor_copy(cand16[:], gidxf[:])
        evr16 = main.tile([128, SLOTS * 128], I16, tag="evr16")
        nc.gpsimd.local_scatter(
            evr16[:], cand16[:], rank16[:],
            channels=128, num_elems=SLOTS * 128, num_idxs=SLOTS * 8,
        )
        evrf = scr.tile([128, SLOTS * 128], F32, tag="evrf")
        nc.vector.tensor_copy(evrf[:], evr16[:])
        allidx_ps = psum_sm.tile([1, SLOTS * 128], F32, tag="smwide")
        nc.tensor.matmul(allidx_ps[:, 0:512], ones_p[:], evrf[:, 0:512])
        nc.tensor.matmul(
            allidx_ps[:, 512 : SLOTS * 128], ones_p[:], evrf[:, 512 : SLOTS * 128]
        )
        allidx = main.tile([1, SLOTS * 128], F32, tag="allidx")
        nc.scalar.copy(allidx[:], allidx_ps[:])
        widx_ps = psum_sm.tile([128, 56], F32, tag="sm")
        av = allidx[:].rearrange("p (s m) -> p m s", m=16)
        for m in range(16):
            nc.tensor.matmul(
                widx_ps[:],
                rep16f_sb[0:1, 128 * m : 128 * (m + 1)],
                av[0:1, m, :],
                start=(m == 0),
                stop=(m == 15),
            )
        # [idx | idx + B] int16 (max 16383 + 16384 = 32767 fits)
        idxs_i16 = main.tile([128, 112], I16, tag="idxs")
        nc.vector.tensor_copy(idxs_i16[:, 0:56], widx_ps[:])
        nc.vector.tensor_scalar_add(idxs_i16[:, 56:112], widx_ps[:], float(B))
        if "stopB" in variant:
            _stop(allidx[0:1, :])
            return

        # ---- stage D: one gather for all 1792 rows ---------------------
        # g[p, n, :] = z[idx[128n + p]]; n = s + 7h exactly matches the
        # (slot, half) block order used everywhere below.
        g = main.tile([128, NBLK, 128], F32, tag="g")
        for hh in range(2):
            nc.gpsimd.dma_gather(
                g[:, 7 * hh : 7 * hh + 7, :],
                z.ap(),
                idxs_i16[:, 56 * hh : 56 * hh + 56],
                num_idxs=SLOTS * 128, num_idxs_reg=SLOTS * 128, elem_size=D,
            )

        if "stopD" in variant:
            _stop(g[0:1, 0, :])
            return

        # ---- stage E: normalize + transpose into bf16 table ------------
        sq = scr.tile([128, NBLK, 128], F32, tag="sq")
        nc.vector.tensor_tensor(sq[:], g[:], g[:], op=ALU.mult)
        sqs = main.tile([128, NBLK], F32, tag="sqs")
        nc.vector.tensor_reduce(
            sqs[:], sq[:], axis=mybir.AxisListType.X, op=ALU.add
        )
        lnv = scr.tile([128, NBLK], F32, tag="lnv")
        nc.scalar.activation(lnv[:], sqs[:], AF.Ln)
        rnw = main.tile([128, NBLK], F32, tag="rnw")
        nc.scalar.activation(rnw[:], lnv[:], AF.Exp, scale=-0.5)
        nc.vector.tensor_tensor(rnw[:], rnw[:], wrow_sb, op=ALU.mult)

        agin = [
            dram.tile([128, CH_SZ[k]], BF16, name=f"agin{k}") for k in range(2)
        ]
        agout = [
            dram.tile(
                [N_CORES * 128, CH_SZ[k]], BF16, addr_space="Shared",
                name=f"agout{k}",
            )
            for k in range(2)
        ]
        flatT = main.tile([128, TBL], BF16, tag="flatT")
        ag_fired = 0
        for bb in range(NBLK):
            s, h = bb // 2, bb % 2
            n = s + 7 * h
            diag = scr.tile([128, 128], F32, tag="diag")
            nc.vector.tensor_tensor(
                diag[:], ident_sb, rnw[:, n : n + 1].to_broadcast([128, 128]),
                op=ALU.mult,
            )
            tp_ps = psum_tp.tile([128, 128], F32, tag="tp")
            nc.tensor.matmul(tp_ps[:], g[:, n, :], diag[:])
            nc.scalar.copy(
                flatT[:, 256 * s + 128 * h : 256 * s + 128 * h + 128], tp_ps[:]
            )
            while ag_fired < 2 and bb + 1 == CH_DONE_N[ag_fired]:
                k = ag_fired
                if "noag" not in variant:
                    nc.sync.dma_start(
                        agin[k][:], flatT[:, CH_OFF[k] : CH_OFF[k] + CH_SZ[k]]
                    )
                    nc.gpsimd.collective_compute(
                        "AllGather",
                        ALU.bypass,
                        replica_groups=[list(range(N_CORES))],
                        ins=[agin[k].opt()],
                        outs=[agout[k].opt()],
                    )
                ag_fired += 1

        if "stopE" in variant:
            ef = scr.tile([1, LOCAL], F32, tag="ef")
            nc.vector.tensor_copy(ef[:], flatT[0:1, 0:LOCAL])
            _stop(ef[0:1, :])
            return

        # ---- stage F: rotated reload of peer chunks --------------------
        if "noag" not in variant:
            pid = nc.partition_id()
            for k in range(2):
                for j in range(1, N_CORES):
                    rj = (pid + j) & 7
                    dst = GOFF[k] + (j - 1) * CH_SZ[k]
                    nc.sync.dma_start(
                        flatT[:, dst : dst + CH_SZ[k]],
                        agout[k][bass.ds(rj * 128, 128), :],
                    )
        else:
            for k in range(2):
                for j in range(1, N_CORES):
                    dst = GOFF[k] + (j - 1) * CH_SZ[k]
                    nc.sync.dma_start(
                        flatT[:, dst : dst + CH_SZ[k]],
                        flatT[:, CH_OFF[k] : CH_OFF[k] + CH_SZ[k]],
                    )

        # ---- stage G: sim sweep (supertile-major) ----------------------
        setup_ctx.close()
        psum_sim = ctx.enter_context(
            tc.tile_pool(name=R + "psum_sim", bufs=2, space="PSUM")
        )
        if "nosweep" in variant:
            nos = main.tile([1, 1], F32, tag="nos")
            nc.vector.tensor_reduce(
                nos[:], flatT[0:1, :], axis=mybir.AxisListType.X, op=ALU.add
            )
            nc.sync.dma_start(out[:], nos[:])
            return
        partials = main.tile([128, NBLK, NST], F32, tag="partials")
        own_t = main.tile([128, NBLK], F32, tag="own_t")
        pos_t = main.tile([128, NBLK], F32, tag="pos_t")
        for st in range(NST):
            for n in range(NBLK):
                s, h = n % 7, n // 7
                lhsT = flatT[:, 256 * s + 128 * h : 256 * s + 128 * h + 128]
                sim_ps = psum_sim.tile([128, ST], F32, tag="sim")
                for q0 in range(0, ST, MM_CHUNK):
                    nc.tensor.matmul(
                        sim_ps[:, q0 : q0 + MM_CHUNK],
                        lhsT,
                        flatT[:, ST * st + q0 : ST * st + q0 + MM_CHUNK],
                    )
                e_sb = escr.tile([128, ST], BF16, tag="e")
                nc.scalar.activation(
                    e_sb[:],
                    sim_ps[:],
                    AF.Exp,
                    scale=1.0 / TEMP,
                    accum_out=partials[:, n, st : st + 1],
                )
                if st == 0:
                    off = 256 * s
                    nc.vector.tensor_reduce(
                        pos_t[:, n : n + 1],
                        e_sb[:, off : off + 128],
                        axis=mybir.AxisListType.X,
                        op=ALU.add,
                    )
                    nc.vector.tensor_reduce(
                        own_t[:, n : n + 1],
                        e_sb[:, off : off + 256],
                        axis=mybir.AxisListType.X,
                        op=ALU.add,
                    )

        # ---- stage H: reduce to one scalar -----------------------------
        totals = main.tile([128, NBLK], F32, tag="totals")
        nc.vector.tensor_reduce(
            totals[:], partials[:], axis=mybir.AxisListType.X, op=ALU.add
        )
        neg = scr.tile([128, NBLK], F32, tag="neg")
        nc.vector.scalar_tensor_tensor(
            neg[:], totals[:], float(-N_DUMMY_COLS), own_t[:],
            op0=ALU.add, op1=ALU.subtract,
        )
        lnn = scr.tile([128, NBLK], F32, tag="lnn")
        lnp = scr.tile([128, NBLK], F32, tag="lnp")
        nc.scalar.activation(lnn[:], neg[:], AF.Ln)
        nc.scalar.activation(lnp[:], pos_t[:], AF.Ln)
        lrows = main.tile([128, NBLK], F32, tag="lrows")
        nc.vector.tensor_sub(lrows[:], lnn[:], lnp[:])
        fin_ps = psum_sim.tile([1, NBLK], F32, tag="sim")
        nc.tensor.matmul(fin_ps[:], ones_p[:], lrows[:])
        fin_sb = main.tile([1, NBLK], F32, tag="fin_sb")
        nc.vector.tensor_tensor(fin_sb[:], fin_ps[:], wfin_sb[:], op=ALU.mult)
        out_sb = main.tile([1, 1], F32, tag="out_sb")
        nc.vector.tensor_reduce(
            out_sb[:], fin_sb[:], axis=mybir.AxisListType.X, op=ALU.add
        )
        nc.vector.tensor_scalar_mul(out_sb[:], out_sb[:], 1.0 / (2 * K * C))
        nc.sync.dma_start(out[:], out_sb[:])


def _per_core_inputs(prob, z_i, z_j):
    bigc_base, rep16flat = _host_constants()
    zcat = np.ascontiguousarray(np.concatenate([z_i, z_j], axis=0))
    maps = []
    for k in range(N_CORES):
        ncl = CCNT[k]
        cols = list(range(CBASE[k], CBASE[k] + ncl))
        cols = cols + [CBASE[k]] * (SLOTS - ncl)  # dummy slots reuse first col
        # probw[p, 128c+f] = prob[128p+f, cols[c]]
        pw = np.ascontiguousarray(
            prob[:, cols].T.reshape(SLOTS, 128, 128).transpose(1, 0, 2)
            .reshape(128, SLOTS * 128)
        )
        w = np.array([1.0] * ncl + [0.0] * (SLOTS - ncl), dtype=np.float32)
        wn = np.concatenate([w, w])  # n = s + 7h block order
        bigc = bigc_base.copy()
        bigc[:, BC_WROW : BC_WROW + NBLK] = wn[None, :]
        m = {
            "probw": pw,
            "z": zcat,
            "bigc": bigc,
            "rep16flat": rep16flat,
            "wfin": wn[None, :].astype(np.float32).copy(),
        }
        maps.append(m)
    return maps


def kernel(prob, z_i, z_j):
    if "nc" not in _CACHE:
        _CACHE["nc"] = _build_program()
    nc = _CACHE["nc"]
    in_maps = _per_core_inputs(
        np.asarray(prob, dtype=np.float32),
        np.asarray(z_i, dtype=np.float32),
        np.asarray(z_j, dtype=np.float32),
    )
    res = run_bass_kernel_spmd(nc, in_maps, list(range(N_CORES)))
    total = np.float32(0.0)
    for r in res.results:
        total += r["partial0"][0, 0]
    return np.asarray(total, dtype=np.float32)
